# revision 1
# baseline (speedup 1.0000x reference)
"""Trainium2 Bass kernel for nn_CustomCrossModalAttention (B=2, N=2048, D=768, H=12).

Sharding (8 cores, zero redundant matmul work):
  - core c owns batch b = c//4 and query rows [512*(c%4), 512*(c%4)+512) of that batch.
  - Phase 1 (row-parallel): each core computes q, k, v projections + LayerNorm for its
    512 rows only. k is folded with the positional term: the reference computes
    scores = (q@k^T)*scale + q@pos^T == scale * (q @ (k + pos/scale)^T), so we build
    k' = LN_k(xk) + pos/scale once.
  - Two AllGathers per 4-core batch group exchange the k'^T and v shards (k first so
    score matmuls can start while v is still in flight).
  - Phase 2 (row-parallel): 12-head attention on the core's 512 query rows with
    softmax (exp without max-subtraction; row sums via a ones column appended to v),
    then output proj, gate, fuse and final LayerNorm.

Precision: projections and the output projection run in float32r (TF32-like, same
PE throughput as bf16 at moving-dim >= 256); attention internals and the gate run
in bf16; all accumulation fp32.

Algebraic folds done on the host (all exact):
  - LN(v) gain/bias folded into wo / bo (uses sum_m attn[n,m] == 1 post-normalize).
  - q/k LN gain+bias applied during the PE-transpose copy-out (per-partition scalars
    in the transposed layout).
  - All matmul biases applied as an extra K=1 matmul with a ones row.
"""

import numpy as np
import ml_dtypes

B, N, D = 2, 2048, 768
H, DH = 12, 64
P = 128
CORES, GROUP = 8, 4
S = 512            # query rows per core
NCH = S // P       # 4 row chunks per core
MCH = N // P       # 16 key chunks
G6 = D // P        # 6
SCALE = DH ** -0.5
EPS = 1e-5

BF = ml_dtypes.bfloat16

_CACHE = {}


def _build():
    from contextlib import ExitStack

    import concourse.bacc as bacc
    import concourse.mybir as mybir
    import concourse.tile as tile
    from concourse.masks import make_identity

    f32 = mybir.dt.float32
    f32r = mybir.dt.float32r
    bf16 = mybir.dt.bfloat16
    ALU = mybir.AluOpType
    ACTF = mybir.ActivationFunctionType

    nc = bacc.Bacc("TRN2", target_bir_lowering=False, num_devices=CORES)

    def din(name, shape, dt=bf16):
        return nc.dram_tensor(name, shape, dt, kind="ExternalInput")

    xqT = din("xqT", [D, S], f32r)      # infrared rows, transposed
    xvT = din("xvT", [D, S], f32r)      # visible rows, transposed
    vis_nat = din("vis_nat", [S, D], f32)
    posTb = din("posTb", [D, S])        # pos/scale + lnk_b, transposed (bf16)
    wqkvT = din("wqkvT", [D, 3 * D], f32r)
    bqkv = din("bqkv", [1, 3 * D], f32r)
    woT = din("woT", [D, D], f32r)      # (wo * lnv_w).T
    bo_a = din("bo_a", [1, D], f32r)    # bo + wo @ lnv_b
    gwT = din("gwT", [2 * D, D], f32r)
    gb = din("gb", [1, D], f32r)
    lnq_g = din("lnq_g", [P, G6], f32)
    lnq_b = din("lnq_b", [P, G6], f32)
    lnk_g = din("lnk_g", [P, G6], f32)
    lnf = din("lnf", [2, D], f32)
    out_rows = nc.dram_tensor("out_rows", [S, D], f32, kind="ExternalOutput")

    FLK = D * S                      # k'^T payload
    FLV = NCH * P * H * (DH + 1)     # v payload (padded with ones col)
    cc_in_k = nc.dram_tensor("cc_in_k", [FLK], f32r)
    cc_out_k = nc.dram_tensor("cc_out_k", [GROUP, FLK], f32r)
    cc_in_v = nc.dram_tensor("cc_in_v", [FLV], bf16)
    cc_out_v = nc.dram_tensor("cc_out_v", [GROUP, FLV], bf16)
    groups = [[0, 1, 2, 3], [4, 5, 6, 7]]

    HALves = [(0, 512), (512, D)]

    with tile.TileContext(nc) as tc, ExitStack() as ctx:
        const = ctx.enter_context(tc.tile_pool(name="const", bufs=1))
        persist = ctx.enter_context(tc.tile_pool(name="persist", bufs=1))

        ident = const.tile([P, P], bf16)
        make_identity(nc, ident)
        ident_f32 = const.tile([P, P], f32)
        make_identity(nc, ident_f32)
        ones_r_f = const.tile([1, P], f32)
        nc.vector.memset(ones_r_f, 1.0)
        ones_r = ones_r_f.bitcast(f32r)
        ones_bf = const.tile([1, P], bf16)
        nc.vector.memset(ones_bf, 1.0)
        ones_f32 = const.tile([1, P], f32)
        nc.vector.memset(ones_f32, 1.0)
        eps_t = const.tile([P, 1], f32)
        nc.vector.memset(eps_t, EPS)

        xvT_sb = const.tile([P, G6, S], f32r)
        nc.gpsimd.dma_start(out=xvT_sb, in_=xvT.rearrange("(s p) n -> p s n", p=P))
        woT_sb = const.tile([P, G6, D], f32r)
        nc.scalar.dma_start(out=woT_sb, in_=woT.rearrange("(s p) o -> p s o", p=P))
        bo_sb = const.tile([1, D], f32r)
        nc.sync.dma_start(out=bo_sb, in_=bo_a.ap())
        gb_sb = const.tile([1, D], f32r)
        nc.sync.dma_start(out=gb_sb, in_=gb.ap())
        lnq_g_sb = const.tile([P, G6], f32)
        nc.sync.dma_start(out=lnq_g_sb, in_=lnq_g.ap())
        lnq_b_sb = const.tile([P, G6], f32)
        nc.sync.dma_start(out=lnq_b_sb, in_=lnq_b.ap())
        lnk_g_sb = const.tile([P, G6], f32)
        nc.sync.dma_start(out=lnk_g_sb, in_=lnk_g.ap())
        lnfw_sb = const.tile([1, D], f32)
        nc.sync.dma_start(out=lnfw_sb, in_=lnf.ap()[0:1, :])
        lnfb_sb = const.tile([1, D], f32)
        nc.sync.dma_start(out=lnfb_sb, in_=lnf.ap()[1:2, :])

        outT_sb = persist.tile([P, G6, S], f32r)

        with tc.tile_pool(name="mid", bufs=1) as midp:
            qT_sb = midp.tile([P, G6, S], f32r)

            with (
                tc.tile_pool(name="ph1", bufs=1) as ph1,
                tc.tile_pool(name="wrot", bufs=2) as wrot,
                tc.tile_pool(name="pwork", bufs=1) as pwork,
                tc.tile_pool(name="stat", bufs=6) as stat,
                tc.tile_pool(name="psum_p", bufs=2, space="PSUM") as psum_p,
                tc.tile_pool(name="psum_t", bufs=2, space="PSUM") as psum_t,
            ):
                xqT_sb = ph1.tile([P, G6, S], f32r)
                nc.gpsimd.dma_start(
                    out=xqT_sb, in_=xqT.rearrange("(s p) n -> p s n", p=P)
                )
                bqkv_sb = ph1.tile([1, 3 * D], f32r)
                nc.sync.dma_start(out=bqkv_sb, in_=bqkv.ap())
                posTb_sb = ph1.tile([P, G6, S], bf16)
                nc.sync.dma_start(
                    out=posTb_sb, in_=posTb.rearrange("(s p) n -> p s n", p=P)
                )

                kloc_sb = ph1.tile([P, G6, S], f32r)   # local k'^T shard
                vloc_sb = ph1.tile([P, NCH, H, DH + 1], bf16)
                nc.vector.memset(vloc_sb[:, :, :, DH:DH + 1], 1.0)

                def load_w(off):
                    w_sb = wrot.tile([P, G6, D], f32r, tag="w")
                    nc.sync.dma_start(
                        out=w_sb,
                        in_=wqkvT.rearrange("(s p) o -> p s o", p=P)[:, :, off:off + D],
                    )
                    return w_sb

                def proj_tile(lhsT_sb, w_sb, w_off, c):
                    py = psum_p.tile([P, D], f32)
                    for o0, o1 in HALves:
                        for s in range(G6):
                            nc.tensor.matmul(
                                py[:, o0:o1],
                                lhsT_sb[:, s, c * P:(c + 1) * P],
                                w_sb[:, s, o0:o1],
                                start=(s == 0), stop=False,
                            )
                        nc.tensor.matmul(
                            py[:, o0:o1], ones_r,
                            bqkv_sb[:, w_off + o0:w_off + o1],
                            start=False, stop=True,
                        )
                    return py

                def ln_stats(y, pool):
                    st = pool.tile([P, 2, 6], f32)
                    for i in range(2):
                        nc.vector.bn_stats(
                            out=st[:, i], in_=y[:, i * 384:(i + 1) * 384]
                        )
                    mv = pool.tile([P, 2], f32)
                    nc.vector.bn_aggr(out=mv, in_=st)
                    rstd = pool.tile([P, 1], f32)
                    nc.scalar.activation(
                        out=rstd, in_=mv[:, 1:2], func=ACTF.Sqrt,
                        bias=eps_t, scale=1.0,
                    )
                    nc.vector.reciprocal(out=rstd, in_=rstd)
                    # negmr = -mu*rstd: ACT applies (y-mu)*rstd as y*rstd+negmr
                    negmr = pool.tile([P, 1], f32)
                    nc.vector.tensor_scalar(
                        out=negmr, in0=mv[:, 0:1], scalar1=rstd, scalar2=-1.0,
                        op0=ALU.mult, op1=ALU.mult,
                    )
                    return negmr, rstd

                # ---- k' ----
                wk_sb = load_w(D)
                knats = []
                for c in range(NCH):
                    py = proj_tile(xvT_sb, wk_sb, D, c)
                    negmr, rstd = ln_stats(py, stat)
                    knat = pwork.tile([P, D], f32, tag=f"knat{c}")
                    nc.scalar.activation(
                        out=knat, in_=py, func=ACTF.Identity,
                        bias=negmr, scale=rstd,
                    )
                    knats.append(knat)
                for s in range(G6):
                    pt = psum_t.tile([P, NCH, P], f32)
                    for c in range(NCH):
                        nc.tensor.transpose(
                            pt[:, c], knats[c][:, s * P:(s + 1) * P], ident_f32
                        )
                    nc.vector.scalar_tensor_tensor(
                        out=kloc_sb[:, s, :],
                        in0=pt.rearrange("p c n -> p (c n)"),
                        scalar=lnk_g_sb[:, s:s + 1],
                        in1=posTb_sb[:, s, :],
                        op0=ALU.mult, op1=ALU.add,
                    )

                # ---- exchange k' (scores need it first) ----
                nc.sync.dma_start(
                    out=cc_in_k.ap().rearrange("(s p n) -> p s n", p=P, s=G6),
                    in_=kloc_sb,
                )
                nc.gpsimd.collective_compute(
                    "AllGather", ALU.bypass, replica_groups=groups,
                    ins=[cc_in_k.ap().opt()], outs=[cc_out_k.ap().opt()],
                )
                # ---- v ----
                wv_sb = load_w(2 * D)
                for c in range(NCH):
                    py = proj_tile(xvT_sb, wv_sb, 2 * D, c)
                    negmr, rstd = ln_stats(py, stat)
                    nc.scalar.activation(
                        out=vloc_sb[:, c, :, 0:DH],
                        in_=py.rearrange("p (h d) -> p h d", h=H),
                        func=ACTF.Identity, bias=negmr, scale=rstd,
                    )

                nc.sync.dma_start(
                    out=cc_in_v.ap().rearrange("(c p f) -> p c f", c=NCH, p=P),
                    in_=vloc_sb.rearrange("p c h d -> p c (h d)"),
                )
                nc.gpsimd.collective_compute(
                    "AllGather", ALU.bypass, replica_groups=groups,
                    ins=[cc_in_v.ap().opt()], outs=[cc_out_v.ap().opt()],
                )
                # ---- q ----
                wq_sb = load_w(0)
                qnats = []
                for c in range(NCH):
                    py = proj_tile(xqT_sb, wq_sb, 0, c)
                    negmr, rstd = ln_stats(py, stat)
                    qnat = pwork.tile([P, D], f32, tag=f"qnat{c}")
                    nc.scalar.activation(
                        out=qnat, in_=py, func=ACTF.Identity,
                        bias=negmr, scale=rstd,
                    )
                    qnats.append(qnat)
                for s in range(G6):
                    pt = psum_t.tile([P, NCH, P], f32)
                    for c in range(NCH):
                        nc.tensor.transpose(
                            pt[:, c], qnats[c][:, s * P:(s + 1) * P], ident_f32
                        )
                    nc.vector.scalar_tensor_tensor(
                        out=qT_sb[:, s, :],
                        in0=pt.rearrange("p c n -> p (c n)"),
                        scalar=lnq_g_sb[:, s:s + 1],
                        in1=lnq_b_sb[:, s:s + 1].to_broadcast([P, S]),
                        op0=ALU.mult, op1=ALU.add,
                    )

            # ---- attention ----
            with (
                tc.tile_pool(name="gath", bufs=1) as gath,
                tc.tile_pool(name="attn", bufs=3) as apool,
                tc.tile_pool(name="hwork", bufs=4) as hwork,
                tc.tile_pool(name="ps_s", bufs=2, space="PSUM") as ps_s,
                tc.tile_pool(name="ps_o", bufs=2, space="PSUM") as ps_o,
            ):
                kT_sb = gath.tile([P, G6, GROUP, S], f32r)      # gathered k'^T
                vaug_sb = gath.tile([P, MCH, H, DH + 1], bf16)  # gathered v + ones
                for g in range(GROUP):
                    eng = nc.sync if g % 2 == 0 else nc.scalar
                    eng.dma_start(
                        out=kT_sb[:, :, g, :],
                        in_=cc_out_k[g:g + 1, :].rearrange(
                            "x (s p n) -> (x p) s n", p=P, s=G6
                        ),
                    )
                for g in range(GROUP):
                    eng = nc.sync if g % 2 == 0 else nc.scalar
                    eng.dma_start(
                        out=vaug_sb[:, 4 * g:4 * g + 4, :, :].rearrange(
                            "p c h d -> p c (h d)"
                        ),
                        in_=cc_out_v[g:g + 1, :].rearrange(
                            "x (c p f) -> (x p) c f", c=NCH, p=P
                        ),
                    )
                for h in range(H):
                    p0 = DH * (h % 2)
                    grp = h // 2
                    po = ps_o.tile([DH + 1, S], f32)
                    for mc0, w in ((0, 3), (3, 3), (6, 3), (9, 3), (12, 3), (15, 1)):
                        ps = ps_s.tile([P, 3, S], f32, tag="ps3")
                        for j in range(w):
                            mc = mc0 + j
                            nc.tensor.matmul(
                                ps[:, j],
                                kT_sb[p0:p0 + DH, grp, mc // 4,
                                      (mc % 4) * P:(mc % 4 + 1) * P],
                                qT_sb[p0:p0 + DH, grp, :],
                                start=True, stop=True,
                            )
                        at = apool.tile([P, 3, S], bf16, tag="at")
                        nc.scalar.activation(
                            out=at[:, :w], in_=ps[:, :w], func=ACTF.Exp, scale=SCALE
                        )
                        for j in range(w):
                            mc = mc0 + j
                            nc.tensor.matmul(
                                po, vaug_sb[:, mc, h, :], at[:, j],
                                start=(mc == 0), stop=(mc == MCH - 1),
                            )
                    rinv = hwork.tile([1, S], f32, tag="rinv")
                    nc.vector.reciprocal(out=rinv, in_=po[DH:DH + 1, :])
                    rbc = hwork.tile([DH, S], f32, tag="rbc")
                    nc.gpsimd.partition_broadcast(rbc, rinv)
                    nc.vector.tensor_tensor(
                        out=outT_sb[p0:p0 + DH, grp, :], in0=po[0:DH, :],
                        in1=rbc, op=ALU.mult,
                    )

        # ---- output proj, gate, fuse, final LN ----
        with (
            tc.tile_pool(name="zpool", bufs=1) as zpool,
            tc.tile_pool(name="fwork", bufs=2) as fwork,
            tc.tile_pool(name="stat2", bufs=6) as stat2,
            tc.tile_pool(name="ps_z", bufs=2, space="PSUM") as ps_z,
            tc.tile_pool(name="ps_t2", bufs=2, space="PSUM") as ps_t2,
        ):
            vis_sb = zpool.tile([P, NCH, D], f32)
            nc.gpsimd.dma_start(
                out=vis_sb, in_=vis_nat.rearrange("(c p) o -> p c o", p=P)
            )
            gwT_sb = zpool.tile([P, 2 * G6, D], f32r)
            nc.scalar.dma_start(
                out=gwT_sb, in_=gwT.rearrange("(s p) o -> p s o", p=P)
            )
            z_sb = zpool.tile([P, NCH, D], f32)
            zT_sb = zpool.tile([P, G6, S], f32r)
            gbc = zpool.tile([P, D], f32)
            bbc = zpool.tile([P, D], f32)

            # broadcast final-LN gain/bias across partitions via K=1 matmul
            for dst, src_row in ((gbc, lnfw_sb), (bbc, lnfb_sb)):
                pb = ps_z.tile([P, D], f32, tag="pz")
                for o0, o1 in HALves:
                    nc.tensor.matmul(
                        pb[:, o0:o1], ones_f32, src_row[:, o0:o1],
                        start=True, stop=True,
                    )
                nc.vector.tensor_copy(out=dst, in_=pb)

            def ln_stats2(y):
                st = stat2.tile([P, 2, 6], f32)
                for i in range(2):
                    nc.vector.bn_stats(out=st[:, i], in_=y[:, i * 384:(i + 1) * 384])
                mv = stat2.tile([P, 2], f32)
                nc.vector.bn_aggr(out=mv, in_=st)
                rstd = stat2.tile([P, 1], f32)
                nc.scalar.activation(
                    out=rstd, in_=mv[:, 1:2], func=ACTF.Sqrt, bias=eps_t, scale=1.0
                )
                nc.vector.reciprocal(out=rstd, in_=rstd)
                negmr = stat2.tile([P, 1], f32)
                nc.vector.tensor_scalar(
                    out=negmr, in0=mv[:, 0:1], scalar1=rstd, scalar2=-1.0,
                    op0=ALU.mult, op1=ALU.mult,
                )
                return negmr, rstd

            for c in range(NCH):
                pz = ps_z.tile([P, D], f32, tag="pz")
                for o0, o1 in HALves:
                    for s in range(G6):
                        nc.tensor.matmul(
                            pz[:, o0:o1],
                            outT_sb[:, s, c * P:(c + 1) * P],
                            woT_sb[:, s, o0:o1],
                            start=(s == 0), stop=False,
                        )
                    nc.tensor.matmul(
                        pz[:, o0:o1], ones_r, bo_sb[:, o0:o1],
                        start=False, stop=True,
                    )
                nc.scalar.copy(out=z_sb[:, c], in_=pz)
            for s in range(G6):
                pt = ps_t2.tile([P, NCH, P], f32)
                for c in range(NCH):
                    nc.tensor.transpose(
                        pt[:, c], z_sb[:, c, s * P:(s + 1) * P], ident_f32
                    )
                nc.scalar.copy(
                    out=zT_sb[:, s, :], in_=pt.rearrange("p c n -> p (c n)")
                )

            gsigs = []
            for c in range(NCH):
                pg = ps_z.tile([P, D], f32, tag="pz")
                for o0, o1 in HALves:
                    for s in range(G6):
                        nc.tensor.matmul(
                            pg[:, o0:o1],
                            xvT_sb[:, s, c * P:(c + 1) * P],
                            gwT_sb[:, s, o0:o1],
                            start=(s == 0), stop=False,
                        )
                    for s in range(G6):
                        nc.tensor.matmul(
                            pg[:, o0:o1],
                            zT_sb[:, s, c * P:(c + 1) * P],
                            gwT_sb[:, G6 + s, o0:o1],
                            start=False, stop=False,
                        )
                    nc.tensor.matmul(
                        pg[:, o0:o1], ones_r, gb_sb[:, o0:o1],
                        start=False, stop=True,
                    )
                gsig = zpool.tile([P, D], bf16, tag=f"gsig{c}")
                nc.scalar.activation(out=gsig, in_=pg, func=ACTF.Sigmoid)
                gsigs.append(gsig)

            for c in range(NCH):
                gsig = gsigs[c]
                dvz = fwork.tile([P, D], f32, tag="dvz")
                nc.gpsimd.tensor_tensor(
                    out=dvz, in0=vis_sb[:, c], in1=z_sb[:, c], op=ALU.subtract
                )
                fus = fwork.tile([P, D], f32, tag="fus")
                nc.vector.tensor_tensor(out=fus, in0=gsig, in1=dvz, op=ALU.mult)
                nc.vector.tensor_tensor(out=fus, in0=fus, in1=z_sb[:, c], op=ALU.add)
                negmr, rstd = ln_stats2(fus)
                tnorm = fwork.tile([P, D], f32, tag="tnorm")
                nc.scalar.activation(
                    out=tnorm, in_=fus, func=ACTF.Identity, bias=negmr, scale=rstd
                )
                nc.vector.tensor_tensor(out=tnorm, in0=tnorm, in1=gbc, op=ALU.mult)
                nc.vector.tensor_tensor(out=tnorm, in0=tnorm, in1=bbc, op=ALU.add)
                nc.sync.dma_start(
                    out=out_rows.rearrange("(c p) o -> p c o", p=P)[:, c], in_=tnorm
                )

    nc.compile()
    return nc


def _prepare_in_maps(inputs):
    f32 = np.float32
    vis = np.asarray(inputs["visible_features"], f32)
    inf = np.asarray(inputs["infrared_features"], f32)
    wq = np.asarray(inputs["wq"], f32)
    bq = np.asarray(inputs["bq"], f32)
    lnq_w = np.asarray(inputs["lnq_w"], f32)
    lnq_b = np.asarray(inputs["lnq_b"], f32)
    wk = np.asarray(inputs["wk"], f32)
    bk = np.asarray(inputs["bk"], f32)
    lnk_w = np.asarray(inputs["lnk_w"], f32)
    lnk_b = np.asarray(inputs["lnk_b"], f32)
    wv = np.asarray(inputs["wv"], f32)
    bv = np.asarray(inputs["bv"], f32)
    lnv_w = np.asarray(inputs["lnv_w"], f32)
    lnv_b = np.asarray(inputs["lnv_b"], f32)
    pos = np.asarray(inputs["pos_emb"], f32)[:N]
    wo = np.asarray(inputs["wo"], f32)
    bo = np.asarray(inputs["bo"], f32)
    gw = np.asarray(inputs["gate_w"], f32)
    gb_ = np.asarray(inputs["gate_b"], f32)
    ln_w = np.asarray(inputs["ln_w"], f32)
    ln_b = np.asarray(inputs["ln_b"], f32)

    wqkvT = np.ascontiguousarray(np.concatenate([wq.T, wk.T, wv.T], axis=1))
    bqkv = np.ascontiguousarray(np.concatenate([bq, bk, bv])[None])
    woT = np.ascontiguousarray((wo * lnv_w[None, :]).T)   # fold LN_v gain
    bo_a = np.ascontiguousarray((bo + wo @ lnv_b)[None])  # fold LN_v bias
    gwT = np.ascontiguousarray(gw.T)
    gbr = np.ascontiguousarray(gb_[None])
    lnq_g = np.ascontiguousarray(lnq_w.reshape(G6, P).T)
    lnq_b2 = np.ascontiguousarray(lnq_b.reshape(G6, P).T)
    lnk_g = np.ascontiguousarray(lnk_w.reshape(G6, P).T)
    lnf = np.stack([ln_w, ln_b])

    in_maps = []
    for c in range(CORES):
        b, r0 = c // GROUP, (c % GROUP) * S
        in_maps.append({
            "xqT": np.ascontiguousarray(inf[b, r0:r0 + S].T),
            "xvT": np.ascontiguousarray(vis[b, r0:r0 + S].T),
            "vis_nat": np.ascontiguousarray(vis[b, r0:r0 + S]),
            "posTb": np.ascontiguousarray(
                pos[r0:r0 + S].T / SCALE + lnk_b[:, None]
            ).astype(BF),
            "wqkvT": wqkvT,
            "bqkv": bqkv,
            "woT": woT,
            "bo_a": bo_a,
            "gwT": gwT,
            "gb": gbr,
            "lnq_g": lnq_g,
            "lnq_b": lnq_b2,
            "lnk_g": lnk_g,
            "lnf": lnf,
        })
    return in_maps


def kernel(trace=False, **inputs):
    from concourse.bass_utils import run_bass_kernel_spmd

    if "nc" not in _CACHE:
        _CACHE["nc"] = _build()
    nc = _CACHE["nc"]
    in_maps = _prepare_in_maps(inputs)
    res = run_bass_kernel_spmd(
        nc, in_maps, core_ids=list(range(CORES)), trace=trace
    )
    out = np.empty((B, N, D), np.float32)
    for c in range(CORES):
        b, r0 = c // GROUP, (c % GROUP) * S
        out[b, r0:r0 + S] = res.results[c]["out_rows"]
    _CACHE["last_result"] = res
    return out



# revision 17
# speedup vs baseline: 1.7434x; 1.7434x over previous
"""Trainium2 Bass kernel for nn_CustomCrossModalAttention (B=2, N=2048, D=768, H=12).

Sharding (8 cores, ZERO collectives):
  - core c owns batch b = c//4 and query rows [512*(c%4), 512*(c%4)+512).
  - k' and v are computed REDUNDANTLY for all 2048 keys of the core's batch
    (the cost-model prices AllGather at 15us + out_bytes/40GB/s, so the two
    baseline gathers cost 267us -- far more than the +46us of replicated PE
    matmul work).
  - Attention, output proj, gate, fuse, final LN are row-parallel on the
    core's own 512 query rows.

Algebra (all exact, matching the reference):
  - scores*scale + q@pos == scale * (q @ (k + pos/scale)^T); pos term and
    lnk bias folded into kT during the transpose copy-out.
  - LN_v gain/bias folded into wo / bo on the host.
  - rstd = exp(-0.5*ln(var+eps)) so the ACT engine never leaves the
    natural_log_exp table during phase1/attention (exp lives there too).

Engines: PE does all matmuls/transposes in bf16 (1 cyc/row); ACT does exp
(the 12.6M-element softmax exp is its ~95us floor) + q-applies; DVE does
bn_stats/aggr + k/v applies; Pool does transposed-copy-outs (STT with
gain*x+bias), v raw psum->sbuf copies, and the attention division.
"""

import numpy as np
import ml_dtypes

B, N, D = 2, 2048, 768
H, DH = 12, 64
P = 128
CORES, GROUP = 8, 4
S = 512            # query rows per core
NCH = S // P       # 4 own row chunks
MCH = N // P       # 16 key row chunks
G6 = D // P        # 6
SCALE = DH ** -0.5
EPS = 1e-5

BF = ml_dtypes.bfloat16

_CACHE = {}


def _build(has_bqkv, has_bo, has_gb):
    from contextlib import ExitStack

    import concourse.bacc as bacc
    import concourse.mybir as mybir
    import concourse.tile as tile
    from concourse.masks import make_identity

    f32 = mybir.dt.float32
    bf16 = mybir.dt.bfloat16
    ALU = mybir.AluOpType
    ACTF = mybir.ActivationFunctionType

    nc = bacc.Bacc("TRN2", target_bir_lowering=False, num_devices=CORES)

    def din(name, shape, dt=bf16):
        return nc.dram_tensor(name, shape, dt, kind="ExternalInput")

    xqT = din("xqT", [D, S])            # own infrared rows, transposed
    xvTf = din("xvTf", [D, N])          # FULL batch visible rows, transposed
    visT_own = din("visT_own", [D, S])  # own visible rows, transposed (gate)
    vis_nat = din("vis_nat", [S, D], f32)
    posTb = din("posTb", [D, N])        # (pos/scale + lnk_b), transposed
    wqkvT = din("wqkvT", [D, 3 * D])
    woT = din("woT", [D, D])            # (wo * lnv_w).T
    gwT = din("gwT", [2 * D, D])
    lnq_g = din("lnq_g", [P, G6], f32)
    lnq_b = din("lnq_b", [P, G6], f32)
    lnk_g = din("lnk_g", [P, G6], f32)
    lnf = din("lnf", [2, D], f32)
    bqkv = din("bqkv", [1, 3 * D]) if has_bqkv else None
    bo_a = din("bo_a", [1, D]) if has_bo else None
    gb = din("gb", [1, D]) if has_gb else None
    out_rows = nc.dram_tensor("out_rows", [S, D], f32, kind="ExternalOutput")

    POSC = 8                 # posTb streamed in 8 column chunks of 256
    PW = N // POSC           # 256

    with tile.TileContext(nc) as tc, ExitStack() as ctx:
        const = ctx.enter_context(tc.tile_pool(name="const", bufs=1))

        ident = const.tile([P, P], bf16)
        make_identity(nc, ident)
        ident_f32 = const.tile([P, P], f32)
        make_identity(nc, ident_f32)
        eps_t = const.tile([P, 1], f32)
        nc.vector.memset(eps_t, EPS)
        ones_bf = const.tile([1, P], bf16)
        nc.vector.memset(ones_bf, 1.0)
        ones_f32 = const.tile([1, P], f32)
        nc.vector.memset(ones_f32, 1.0)

        lnq_g_sb = const.tile([P, G6], f32)
        nc.sync.dma_start(out=lnq_g_sb, in_=lnq_g.ap())
        lnq_b_sb = const.tile([P, G6], f32)
        nc.sync.dma_start(out=lnq_b_sb, in_=lnq_b.ap())
        lnk_g_sb = const.tile([P, G6], f32)
        nc.sync.dma_start(out=lnk_g_sb, in_=lnk_g.ap())
        lnfw_sb = const.tile([1, D], f32)
        nc.sync.dma_start(out=lnfw_sb, in_=lnf.ap()[0:1, :])
        lnfb_sb = const.tile([1, D], f32)
        nc.sync.dma_start(out=lnfb_sb, in_=lnf.ap()[1:2, :])

        # long-lived activation tensors
        qT_sb = const.tile([P, G6, S], bf16)
        kT_sb = const.tile([P, G6, N], bf16)
        outT_sb = const.tile([P, G6, S], bf16)

        if has_bqkv:
            bqkv_sb = const.tile([1, 3 * D], bf16)
            nc.scalar.dma_start(out=bqkv_sb, in_=bqkv.ap())

        HALves = [(0, 512), (512, D)]

        def proj_chunk(py, lhsT_sb, w_sb, w_off, c):
            """py[128,768] = x-chunk-c @ W (+bias via ones row when present)."""
            for o0, o1 in HALves:
                for s in range(G6):
                    nc.tensor.matmul(
                        py[:, o0:o1],
                        lhsT_sb[:, s, c * P:(c + 1) * P],
                        w_sb[:, s, o0:o1],
                        start=(s == 0), stop=(not has_bqkv and s == G6 - 1),
                    )
                if has_bqkv:
                    nc.tensor.matmul(
                        py[:, o0:o1], ones_bf,
                        bqkv_sb[:, w_off + o0:w_off + o1],
                        start=False, stop=True,
                    )

        def stats(py, pool):
            """DVE bn_stats -> (mean, var) [P,2]."""
            st = pool.tile([P, 2, 6], f32, tag="st")
            for i in range(2):
                nc.vector.bn_stats(out=st[:, i], in_=py[:, i * 384:(i + 1) * 384])
            mv = pool.tile([P, 2], f32, tag="mv")
            nc.vector.bn_aggr(out=mv, in_=st)
            return mv

        def rstd_of(mv, pool, tag="rs"):
            """rstd = 1/sqrt(var+eps): ACT Sqrt then DVE reciprocal."""
            rstd = pool.tile([P, 1], f32, tag=tag + "r")
            nc.scalar.activation(out=rstd, in_=mv[:, 1:2], func=ACTF.Sqrt,
                                 bias=eps_t, scale=1.0)
            nc.vector.reciprocal(out=rstd, in_=rstd)
            return rstd

        def negmr_of(mv, rstd, pool, tag="nm"):
            negmr = pool.tile([P, 1], f32, tag=tag)
            nc.vector.tensor_scalar(
                out=negmr, in0=mv[:, 0:1], scalar1=rstd, scalar2=-1.0,
                op0=ALU.mult, op1=ALU.mult,
            )
            return negmr

        mid = ctx.enter_context(tc.tile_pool(name="mid", bufs=1))
        xvT_sb = mid.tile([P, G6, N], bf16)
        nc.sync.dma_start(out=xvT_sb, in_=xvTf.rearrange("(s p) n -> p s n", p=P))
        wv_sb = mid.tile([P, G6, D], bf16)
        vaug_sb = mid.tile([P, MCH, H, DH + 1], bf16)
        nc.vector.memset(vaug_sb[:, :, :, DH:DH + 1], 1.0)
        onat_sb = mid.tile([P, NCH, H, DH], bf16)

        # ---------------- phase 1: q, v, k' (all LN sqrt before any exp) --
        with (
            tc.tile_pool(name="ph1", bufs=1) as ph1,
            tc.tile_pool(name="knp", bufs=3) as knp,
            tc.tile_pool(name="post", bufs=2) as postp,
            tc.tile_pool(name="stat", bufs=4) as statp,
            tc.tile_pool(name="ps_p", bufs=2, space="PSUM") as ps_p,
            tc.tile_pool(name="ps_t", bufs=1, space="PSUM") as ps_t,
        ):
            xqT_sb = ph1.tile([P, G6, S], bf16)
            nc.sync.dma_start(out=xqT_sb, in_=xqT.rearrange("(s p) n -> p s n", p=P))
            wq_sb = ph1.tile([P, G6, D], bf16)
            nc.sync.dma_start(
                out=wq_sb,
                in_=wqkvT.rearrange("(s p) o -> p s o", p=P)[:, :, 0:D],
            )
            nc.sync.dma_start(
                out=wv_sb,
                in_=wqkvT.rearrange("(s p) o -> p s o", p=P)[:, :, 2 * D:3 * D],
            )
            wk_sb = ph1.tile([P, G6, D], bf16)
            nc.sync.dma_start(
                out=wk_sb,
                in_=wqkvT.rearrange("(s p) o -> p s o", p=P)[:, :, D:2 * D],
            )
            qnat = ph1.tile([P, NCH, D], bf16)

            # q projection (apply on ACT)
            for c in range(NCH):
                py = ps_p.tile([P, D], f32, tag="py")
                proj_chunk(py, xqT_sb, wq_sb, 0, c)
                mv = stats(py, statp)
                rstd = rstd_of(mv, statp, "qr")
                negmr = negmr_of(mv, rstd, statp, "qn")
                nc.scalar.activation(
                    out=qnat[:, c], in_=py, func=ACTF.Identity,
                    bias=negmr, scale=rstd,
                )

            # v projection: normalize straight from psum into vaug (DVE)
            for c in range(MCH):
                pv = ps_p.tile([P, D], f32, tag="py")
                proj_chunk(pv, xvT_sb, wv_sb, 2 * D, c)
                mv = stats(pv, statp)
                rstd = rstd_of(mv, statp, "vr")
                negmr = negmr_of(mv, rstd, statp, "vn")
                nc.vector.tensor_scalar(
                    out=vaug_sb[:, c, :, 0:DH],
                    in0=pv.rearrange("p (h d) -> p h d", h=H),
                    scalar1=rstd, scalar2=negmr,
                    op0=ALU.mult, op1=ALU.add,
                )
                if c == 1:
                    # q transpose + STT while v streams (qnat ready)
                    for g in range(2):
                        ptq = ps_t.tile([P, G6, 2 * P], bf16, tag="pt")
                        for i in range(2):
                            qc = g * 2 + i
                            for s in range(G6):
                                nc.tensor.transpose(
                                    ptq[:, s, i * P:(i + 1) * P],
                                    qnat[:, qc, s * P:(s + 1) * P], ident,
                                )
                        for s in range(G6):
                            nc.gpsimd.scalar_tensor_tensor(
                                out=qT_sb[:, s, g * 2 * P:(g + 1) * 2 * P],
                                in0=ptq[:, s], scalar=lnq_g_sb[:, s:s + 1],
                                in1=lnq_b_sb[:, s:s + 1].to_broadcast([P, 2 * P]),
                                op0=ALU.mult, op1=ALU.add,
                            )

            # k' projection + transposed copy-out with pos fold
            def kchunk(c, pend):
                py = ps_p.tile([P, D], f32, tag="py")
                proj_chunk(py, xvT_sb, wk_sb, D, c)
                mv = stats(py, statp)
                rstd = rstd_of(mv, statp, "kr")
                negmr = negmr_of(mv, rstd, statp, "kn")
                pair = c // 2
                if c % 2 == 0:
                    pend["t"] = knp.tile([P, 2, D], bf16, tag="knat",
                                         name=f"knat{c}")
                nc.vector.tensor_scalar(
                    out=pend["t"][:, c % 2], in0=py, scalar1=rstd,
                    scalar2=negmr, op0=ALU.mult, op1=ALU.add,
                )
                pend[pair] = pend["t"]

            def ktranspose(pair, pend):
                knat2 = pend.pop(pair)
                pos_sb = postp.tile([P, G6, 2 * P], bf16, tag="pos")
                nc.sync.dma_start(
                    out=pos_sb,
                    in_=posTb.rearrange("(s p) n -> p s n", p=P)[
                        :, :, pair * 2 * P:(pair + 1) * 2 * P],
                )
                pt = ps_t.tile([P, G6, 2 * P], bf16, tag="pt")
                for i in range(2):
                    for s in range(G6):
                        nc.tensor.transpose(
                            pt[:, s, i * P:(i + 1) * P],
                            knat2[:, i, s * P:(s + 1) * P], ident,
                        )
                for s in range(G6):
                    nc.gpsimd.scalar_tensor_tensor(
                        out=kT_sb[:, s, pair * 2 * P:(pair + 1) * 2 * P],
                        in0=pt[:, s], scalar=lnk_g_sb[:, s:s + 1],
                        in1=pos_sb[:, s], op0=ALU.mult, op1=ALU.add,
                    )

            pend = {}
            for c in range(MCH):
                kchunk(c, pend)
                if c >= 3 and c % 2 == 1:
                    ktranspose((c - 1) // 2 - 1, pend)  # one-pair lag
            ktranspose(MCH // 2 - 1, pend)

        # ---------------- attention: scores/exp (ACT-bound) + lagged av ----
        with (
            tc.tile_pool(name="atp", bufs=2) as atp,
            tc.tile_pool(name="rin", bufs=4) as rinp,
            tc.tile_pool(name="ps_s", bufs=2, space="PSUM") as ps_s,
            tc.tile_pool(name="ps_o", bufs=2, space="PSUM") as ps_o,
        ):
            def scores_group(at_h, h, mc0, w):
                p0 = DH * (h % 2)
                grp = h // 2
                ps = ps_s.tile([P, 3, S], f32, tag="ps3")
                for j in range(w):
                    mc = mc0 + j
                    nc.tensor.matmul(
                        ps[:, j],
                        kT_sb[p0:p0 + DH, grp, mc * P:(mc + 1) * P],
                        qT_sb[p0:p0 + DH, grp, :],
                        start=True, stop=True,
                    )
                nc.scalar.activation(
                    out=at_h[:, mc0:mc0 + w, :], in_=ps[:, :w],
                    func=ACTF.Exp, scale=SCALE,
                )

            def av_head(at_h, h):
                po = ps_o.tile([P, NCH, DH + 1], f32, tag="po")
                for qc in range(NCH):
                    for mc in range(MCH):
                        nc.tensor.matmul(
                            po[:, qc],
                            at_h[:, mc, qc * P:(qc + 1) * P],
                            vaug_sb[:, mc, h, :],
                            start=(mc == 0), stop=(mc == MCH - 1),
                        )
                    rinv = rinp.tile([P, 1], f32, tag="rin")
                    nc.vector.reciprocal(out=rinv, in_=po[:, qc, DH:DH + 1])
                    nc.gpsimd.tensor_scalar_mul(
                        out=onat_sb[:, qc, h], in0=po[:, qc, 0:DH],
                        scalar1=rinv,
                    )

            prev = None
            for h in range(H):
                at_h = atp.tile([P, MCH, S], bf16, tag="at", name=f"at{h}")
                for mc0, w in [(0, 3), (3, 3), (6, 3), (9, 3), (12, 3),
                               (15, 1)]:
                    scores_group(at_h, h, mc0, w)
                if prev is not None:
                    av_head(*prev)
                prev = (at_h, h)
            av_head(*prev)

        # transpose attention output for the output projection
        with tc.tile_pool(name="ps_t2", bufs=1, space="PSUM") as ps_t2:
            if True:
                for g in range(2):
                    pt = ps_t2.tile([P, G6, 2 * P], bf16, tag="pt2")
                    for i in range(2):
                        qc = g * 2 + i
                        src = onat_sb[:, qc].rearrange("p h d -> p (h d)")
                        for s in range(G6):
                            nc.tensor.transpose(
                                pt[:, s, i * P:(i + 1) * P],
                                src[:, s * P:(s + 1) * P], ident,
                            )
                    for s in range(G6):
                        nc.gpsimd.tensor_copy(
                            out=outT_sb[:, s, g * 2 * P:(g + 1) * 2 * P],
                            in_=pt[:, s],
                        )

        # ---------------- phase 3: out proj, gate, fuse, final LN ----------
        with (
            tc.tile_pool(name="ph3", bufs=1) as ph3,
            tc.tile_pool(name="fw", bufs=2) as fw,
            tc.tile_pool(name="st3", bufs=4) as st3,
            tc.tile_pool(name="ps_z", bufs=2, space="PSUM") as ps_z,
            tc.tile_pool(name="ps_t3", bufs=1, space="PSUM") as ps_t3,
        ):
            woT_sb = ph3.tile([P, G6, D], bf16)
            nc.sync.dma_start(out=woT_sb, in_=woT.rearrange("(s p) o -> p s o", p=P))
            vis_sb = ph3.tile([P, NCH, D], f32)
            nc.scalar.dma_start(out=vis_sb, in_=vis_nat.rearrange("(c p) o -> p c o", p=P))
            visT_sb = ph3.tile([P, G6, S], bf16)
            nc.scalar.dma_start(out=visT_sb, in_=visT_own.rearrange("(s p) n -> p s n", p=P))
            gwv_sb = ph3.tile([P, G6, D], bf16)
            nc.sync.dma_start(
                out=gwv_sb,
                in_=gwT.rearrange("(s p) o -> p s o", p=P)[0:P, 0:G6, :],
            )
            gwz_sb = ph3.tile([P, G6, D], bf16)
            nc.sync.dma_start(
                out=gwz_sb,
                in_=gwT.rearrange("(g s p) o -> p (g s) o", p=P, g=2)[:, G6:, :],
            )
            if has_gb:
                gb_sb = ph3.tile([1, D], bf16)
                nc.sync.dma_start(out=gb_sb, in_=gb.ap())
            if has_bo:
                bo_sb = ph3.tile([1, D], bf16)
                nc.sync.dma_start(out=bo_sb, in_=bo_a.ap())

            z_sb = ph3.tile([P, NCH, D], f32)
            zT_sb = ph3.tile([P, G6, S], bf16)
            gsig = ph3.tile([P, NCH, D], bf16)
            gbc = ph3.tile([P, D], f32)
            bbc = ph3.tile([P, D], f32)

            # broadcast final-LN gain/bias across partitions via K=1 matmul
            for dst, src_row in ((gbc, lnfw_sb), (bbc, lnfb_sb)):
                pb = ps_z.tile([P, D], f32, tag="pz")
                for o0, o1 in HALves:
                    nc.tensor.matmul(
                        pb[:, o0:o1], ones_f32, src_row[:, o0:o1],
                        start=True, stop=True,
                    )
                nc.vector.tensor_copy(out=dst, in_=pb)

            # z = attout @ woT' (+bo)
            for c in range(NCH):
                pz = ps_z.tile([P, D], f32, tag="pz")
                for o0, o1 in HALves:
                    for s in range(G6):
                        nc.tensor.matmul(
                            pz[:, o0:o1],
                            outT_sb[:, s, c * P:(c + 1) * P],
                            woT_sb[:, s, o0:o1],
                            start=(s == 0), stop=(not has_bo and s == G6 - 1),
                        )
                    if has_bo:
                        nc.tensor.matmul(
                            pz[:, o0:o1], ones_bf, bo_sb[:, o0:o1],
                            start=False, stop=True,
                        )
                nc.scalar.copy(out=z_sb[:, c], in_=pz)

            # zT for the gate matmul
            for g in range(2):
                pt = ps_t3.tile([P, G6, 2 * P], f32, tag="pt3")
                for i in range(2):
                    c = g * 2 + i
                    for s in range(G6):
                        nc.tensor.transpose(
                            pt[:, s, i * P:(i + 1) * P],
                            z_sb[:, c, s * P:(s + 1) * P], ident_f32,
                        )
                for s in range(G6):
                    nc.gpsimd.tensor_copy(
                        out=zT_sb[:, s, g * 2 * P:(g + 1) * 2 * P],
                        in_=pt[:, s],
                    )

            # gate = sigmoid([vis, z] @ gwT (+gb))
            for c in range(NCH):
                pg = ps_z.tile([P, D], f32, tag="pz")
                for o0, o1 in HALves:
                    for s in range(G6):
                        nc.tensor.matmul(
                            pg[:, o0:o1],
                            visT_sb[:, s, c * P:(c + 1) * P],
                            gwv_sb[:, s, o0:o1],
                            start=(s == 0), stop=False,
                        )
                    for s in range(G6):
                        nc.tensor.matmul(
                            pg[:, o0:o1],
                            zT_sb[:, s, c * P:(c + 1) * P],
                            gwz_sb[:, s, o0:o1],
                            start=False,
                            stop=(not has_gb and s == G6 - 1),
                        )
                    if has_gb:
                        nc.tensor.matmul(
                            pg[:, o0:o1], ones_bf, gb_sb[:, o0:o1],
                            start=False, stop=True,
                        )
                nc.scalar.activation(out=gsig[:, c], in_=pg, func=ACTF.Sigmoid)

            # fuse + final LN
            for c in range(NCH):
                dvz = fw.tile([P, D], f32, tag="dvz")
                nc.gpsimd.tensor_tensor(
                    out=dvz, in0=vis_sb[:, c], in1=z_sb[:, c], op=ALU.subtract,
                )
                fus = fw.tile([P, D], f32, tag="fus")
                nc.vector.tensor_tensor(out=fus, in0=gsig[:, c], in1=dvz,
                                        op=ALU.mult)
                nc.vector.tensor_tensor(out=fus, in0=fus, in1=z_sb[:, c],
                                        op=ALU.add)
                mv = stats(fus, st3)
                rstd = rstd_of(mv, st3, "fr")
                negmr = negmr_of(mv, rstd, st3, "fn")
                tnorm = fw.tile([P, D], f32, tag="tn")
                nc.scalar.activation(
                    out=tnorm, in_=fus, func=ACTF.Identity,
                    bias=negmr, scale=rstd,
                )
                nc.vector.tensor_tensor(out=tnorm, in0=tnorm, in1=gbc,
                                        op=ALU.mult)
                nc.gpsimd.tensor_tensor(out=tnorm, in0=tnorm, in1=bbc,
                                        op=ALU.add)
                nc.sync.dma_start(
                    out=out_rows.rearrange("(c p) o -> p c o", p=P)[:, c],
                    in_=tnorm,
                )

    nc.compile()
    return nc


def _prepare_in_maps(inputs):
    f32 = np.float32
    vis = np.asarray(inputs["visible_features"], f32)
    inf = np.asarray(inputs["infrared_features"], f32)
    wq = np.asarray(inputs["wq"], f32)
    bq = np.asarray(inputs["bq"], f32)
    lnq_w = np.asarray(inputs["lnq_w"], f32)
    lnq_b = np.asarray(inputs["lnq_b"], f32)
    wk = np.asarray(inputs["wk"], f32)
    bk = np.asarray(inputs["bk"], f32)
    lnk_w = np.asarray(inputs["lnk_w"], f32)
    lnk_b = np.asarray(inputs["lnk_b"], f32)
    wv = np.asarray(inputs["wv"], f32)
    bv = np.asarray(inputs["bv"], f32)
    lnv_w = np.asarray(inputs["lnv_w"], f32)
    lnv_b = np.asarray(inputs["lnv_b"], f32)
    pos = np.asarray(inputs["pos_emb"], f32)[:N]
    wo = np.asarray(inputs["wo"], f32)
    bo = np.asarray(inputs["bo"], f32)
    gw = np.asarray(inputs["gate_w"], f32)
    gb_ = np.asarray(inputs["gate_b"], f32)
    ln_w = np.asarray(inputs["ln_w"], f32)
    ln_b = np.asarray(inputs["ln_b"], f32)

    wqkvT = np.concatenate([wq.T, wk.T, wv.T], axis=1).astype(BF)
    bqkv = np.concatenate([bq, bk, bv])[None]
    woT = ((wo * lnv_w[None, :]).T).astype(BF)
    bo_a = (bo + wo @ lnv_b)[None]
    gwT = gw.T.astype(BF)
    lnq_g = np.ascontiguousarray(lnq_w.reshape(G6, P).T)
    lnq_b2 = np.ascontiguousarray(lnq_b.reshape(G6, P).T)
    lnk_g = np.ascontiguousarray(lnk_w.reshape(G6, P).T)
    lnf = np.stack([ln_w, ln_b])
    flags = (
        bool(np.any(bqkv)), bool(np.any(bo_a)), bool(np.any(gb_)),
    )

    in_maps = []
    for c in range(CORES):
        b, r0 = c // GROUP, (c % GROUP) * S
        m = {
            "xqT": np.ascontiguousarray(inf[b, r0:r0 + S].T).astype(BF),
            "xvTf": np.ascontiguousarray(vis[b].T).astype(BF),
            "visT_own": np.ascontiguousarray(vis[b, r0:r0 + S].T).astype(BF),
            "vis_nat": np.ascontiguousarray(vis[b, r0:r0 + S]),
            "posTb": np.ascontiguousarray(
                pos.T / SCALE + lnk_b[:, None]
            ).astype(BF),
            "wqkvT": np.ascontiguousarray(wqkvT),
            "woT": np.ascontiguousarray(woT),
            "gwT": np.ascontiguousarray(gwT),
            "lnq_g": lnq_g,
            "lnq_b": lnq_b2,
            "lnk_g": lnk_g,
            "lnf": lnf,
        }
        if flags[0]:
            m["bqkv"] = np.ascontiguousarray(bqkv).astype(BF)
        if flags[1]:
            m["bo_a"] = np.ascontiguousarray(bo_a).astype(BF)
        if flags[2]:
            m["gb"] = np.ascontiguousarray(gb_[None]).astype(BF)
        in_maps.append(m)
    return in_maps, flags


def kernel(trace=False, **inputs):
    from concourse.bass_utils import run_bass_kernel_spmd

    in_maps, flags = _prepare_in_maps(inputs)
    key = ("nc",) + flags
    if key not in _CACHE:
        _CACHE[key] = _build(*flags)
    nc = _CACHE[key]
    _CACHE["nc"] = nc
    res = run_bass_kernel_spmd(
        nc, in_maps, core_ids=list(range(CORES)), trace=trace
    )
    out = np.empty((B, N, D), np.float32)
    for c in range(CORES):
        b, r0 = c // GROUP, (c % GROUP) * S
        out[b, r0:r0 + S] = res.results[c]["out_rows"]
    _CACHE["last_result"] = res
    return out


# revision 19
# speedup vs baseline: 1.8323x; 1.0510x over previous
"""Trainium2 Bass kernel for nn_CustomCrossModalAttention (B=2, N=2048, D=768, H=12).

Sharding (8 cores, ZERO collectives):
  - core c owns batch b = c//4 and query rows [512*(c%4), 512*(c%4)+512).
  - k' and v are computed REDUNDANTLY for all 2048 keys of the core's batch
    (the cost-model prices AllGather at 15us + out_bytes/40GB/s, so the two
    baseline gathers cost 267us -- far more than the +46us of replicated PE
    matmul work).
  - Attention, output proj, gate, fuse, final LN are row-parallel on the
    core's own 512 query rows.

Algebra (all exact, matching the reference):
  - scores*scale + q@pos == scale * (q @ (k + pos/scale)^T); pos term and
    lnk bias folded into kT during the transpose copy-out.
  - LN_v gain/bias folded into wo / bo on the host.
  - rstd = exp(-0.5*ln(var+eps)) so the ACT engine never leaves the
    natural_log_exp table during phase1/attention (exp lives there too).

Engines: PE does all matmuls/transposes in bf16 (1 cyc/row); ACT does exp
(the 12.6M-element softmax exp is its ~95us floor) + q-applies; DVE does
bn_stats/aggr + k/v applies; Pool does transposed-copy-outs (STT with
gain*x+bias), v raw psum->sbuf copies, and the attention division.
"""

import numpy as np
import ml_dtypes

B, N, D = 2, 2048, 768
H, DH = 12, 64
P = 128
CORES, GROUP = 8, 4
S = 512            # query rows per core
NCH = S // P       # 4 own row chunks
MCH = N // P       # 16 key row chunks
G6 = D // P        # 6
SCALE = DH ** -0.5
EPS = 1e-5

BF = ml_dtypes.bfloat16

_CACHE = {}


def _build(has_bqkv, has_bo, has_gb):
    from contextlib import ExitStack

    import concourse.bacc as bacc
    import concourse.mybir as mybir
    import concourse.tile as tile
    from concourse.masks import make_identity

    f32 = mybir.dt.float32
    bf16 = mybir.dt.bfloat16
    ALU = mybir.AluOpType
    ACTF = mybir.ActivationFunctionType

    nc = bacc.Bacc("TRN2", target_bir_lowering=False, num_devices=CORES)

    def din(name, shape, dt=bf16):
        return nc.dram_tensor(name, shape, dt, kind="ExternalInput")

    xqT = din("xqT", [D, S])            # own infrared rows, transposed
    xvTf = din("xvTf", [D, N])          # FULL batch visible rows, transposed
    visT_own = din("visT_own", [D, S])  # own visible rows, transposed (gate)
    vis_nat = din("vis_nat", [S, D], f32)
    posTb = din("posTb", [D, N])        # (pos/scale + lnk_b), transposed
    wqkvT = din("wqkvT", [D, 3 * D])
    woT = din("woT", [D, D])            # (wo * lnv_w).T
    gwT = din("gwT", [2 * D, D])
    lnq_g = din("lnq_g", [P, G6], f32)
    lnq_b = din("lnq_b", [P, G6], f32)
    lnk_g = din("lnk_g", [P, G6], f32)
    lnf = din("lnf", [2, D], f32)
    bqkv = din("bqkv", [1, 3 * D]) if has_bqkv else None
    bo_a = din("bo_a", [1, D]) if has_bo else None
    gb = din("gb", [1, D]) if has_gb else None
    out_rows = nc.dram_tensor("out_rows", [S, D], f32, kind="ExternalOutput")

    POSC = 8                 # posTb streamed in 8 column chunks of 256
    PW = N // POSC           # 256

    with tile.TileContext(nc) as tc, ExitStack() as ctx:
        const = ctx.enter_context(tc.tile_pool(name="const", bufs=1))

        ident = const.tile([P, P], bf16)
        make_identity(nc, ident)
        ident_f32 = const.tile([P, P], f32)
        make_identity(nc, ident_f32)
        eps_t = const.tile([P, 1], f32)
        nc.vector.memset(eps_t, EPS)
        ones_bf = const.tile([1, P], bf16)
        nc.vector.memset(ones_bf, 1.0)
        ones_f32 = const.tile([1, P], f32)
        nc.vector.memset(ones_f32, 1.0)

        lnq_g_sb = const.tile([P, G6], f32)
        nc.scalar.dma_start(out=lnq_g_sb, in_=lnq_g.ap())
        lnq_b_sb = const.tile([P, G6], f32)
        nc.scalar.dma_start(out=lnq_b_sb, in_=lnq_b.ap())
        lnk_g_sb = const.tile([P, G6], f32)
        nc.scalar.dma_start(out=lnk_g_sb, in_=lnk_g.ap())
        lnfw_sb = const.tile([1, D], f32)
        nc.scalar.dma_start(out=lnfw_sb, in_=lnf.ap()[0:1, :])
        lnfb_sb = const.tile([1, D], f32)
        nc.scalar.dma_start(out=lnfb_sb, in_=lnf.ap()[1:2, :])

        # long-lived activation tensors
        qT_sb = const.tile([P, G6, S], bf16)
        kT_sb = const.tile([P, G6, N], bf16)
        outT_sb = const.tile([P, G6, S], bf16)

        if has_bqkv:
            bqkv_sb = const.tile([1, 3 * D], bf16)
            nc.scalar.dma_start(out=bqkv_sb, in_=bqkv.ap())

        HALves = [(0, 512), (512, D)]

        def proj_chunk(py, lhsT_sb, w_sb, w_off, c):
            """py[128,768] = x-chunk-c @ W (+bias via ones row when present)."""
            for o0, o1 in HALves:
                for s in range(G6):
                    nc.tensor.matmul(
                        py[:, o0:o1],
                        lhsT_sb[:, s, c * P:(c + 1) * P],
                        w_sb[:, s, o0:o1],
                        start=(s == 0), stop=(not has_bqkv and s == G6 - 1),
                    )
                if has_bqkv:
                    nc.tensor.matmul(
                        py[:, o0:o1], ones_bf,
                        bqkv_sb[:, w_off + o0:w_off + o1],
                        start=False, stop=True,
                    )

        def stats(py, pool):
            """DVE bn_stats -> (mean, var) [P,2]."""
            st = pool.tile([P, 2, 6], f32, tag="st")
            for i in range(2):
                nc.vector.bn_stats(out=st[:, i], in_=py[:, i * 384:(i + 1) * 384])
            mv = pool.tile([P, 2], f32, tag="mv")
            nc.vector.bn_aggr(out=mv, in_=st)
            return mv

        def rstd_of(mv, pool, tag="rs"):
            """rstd = 1/sqrt(var+eps): ACT Sqrt then DVE reciprocal."""
            rstd = pool.tile([P, 1], f32, tag=tag + "r")
            nc.scalar.activation(out=rstd, in_=mv[:, 1:2], func=ACTF.Sqrt,
                                 bias=eps_t, scale=1.0)
            nc.vector.reciprocal(out=rstd, in_=rstd)
            return rstd

        def negmr_of(mv, rstd, pool, tag="nm"):
            negmr = pool.tile([P, 1], f32, tag=tag)
            nc.vector.tensor_scalar(
                out=negmr, in0=mv[:, 0:1], scalar1=rstd, scalar2=-1.0,
                op0=ALU.mult, op1=ALU.mult,
            )
            return negmr

        mid = ctx.enter_context(tc.tile_pool(name="mid", bufs=1))
        xvT_sb = mid.tile([P, G6, N], bf16)
        wv_sb = mid.tile([P, G6, D], bf16)
        vaug_sb = mid.tile([P, MCH, H, DH + 1], bf16)
        nc.vector.memset(vaug_sb[:, :, :, DH:DH + 1], 1.0)
        onat_sb = mid.tile([P, NCH, H, DH], bf16)

        # ---------------- phase 1: q, v, k' (all LN sqrt before any exp) --
        with (
            tc.tile_pool(name="ph1", bufs=1) as ph1,
            tc.tile_pool(name="knp", bufs=3) as knp,
            tc.tile_pool(name="post", bufs=2) as postp,
            tc.tile_pool(name="stat", bufs=4) as statp,
            tc.tile_pool(name="ps_p", bufs=2, space="PSUM") as ps_p,
            tc.tile_pool(name="ps_t", bufs=1, space="PSUM") as ps_t,
        ):
            xqT_sb = ph1.tile([P, G6, S], bf16)
            nc.sync.dma_start(out=xqT_sb, in_=xqT.rearrange("(s p) n -> p s n", p=P))
            nc.gpsimd.dma_start(out=xvT_sb, in_=xvTf.rearrange("(s p) n -> p s n", p=P))
            wq_sb = ph1.tile([P, G6, D], bf16)
            nc.sync.dma_start(
                out=wq_sb,
                in_=wqkvT.rearrange("(s p) o -> p s o", p=P)[:, :, 0:D],
            )
            nc.sync.dma_start(
                out=wv_sb,
                in_=wqkvT.rearrange("(s p) o -> p s o", p=P)[:, :, 2 * D:3 * D],
            )
            wk_sb = ph1.tile([P, G6, D], bf16)
            nc.sync.dma_start(
                out=wk_sb,
                in_=wqkvT.rearrange("(s p) o -> p s o", p=P)[:, :, D:2 * D],
            )
            qnat = ph1.tile([P, NCH, D], bf16)

            # q projection (apply on ACT)
            for c in range(NCH):
                py = ps_p.tile([P, D], f32, tag="py")
                proj_chunk(py, xqT_sb, wq_sb, 0, c)
                mv = stats(py, statp)
                rstd = rstd_of(mv, statp, "qr")
                negmr = negmr_of(mv, rstd, statp, "qn")
                nc.scalar.activation(
                    out=qnat[:, c], in_=py, func=ACTF.Identity,
                    bias=negmr, scale=rstd,
                )

            # v projection: normalize straight from psum into vaug (DVE)
            for c in range(MCH):
                pv = ps_p.tile([P, D], f32, tag="py")
                proj_chunk(pv, xvT_sb, wv_sb, 2 * D, c)
                mv = stats(pv, statp)
                rstd = rstd_of(mv, statp, "vr")
                negmr = negmr_of(mv, rstd, statp, "vn")
                nc.scalar.activation(
                    out=vaug_sb[:, c, :, 0:DH],
                    in_=pv.rearrange("p (h d) -> p h d", h=H),
                    func=ACTF.Identity, bias=negmr, scale=rstd,
                )
                if c == 1:
                    # q transpose + STT while v streams (qnat ready)
                    for g in range(2):
                        ptq = ps_t.tile([P, G6, 2 * P], bf16, tag="pt")
                        for i in range(2):
                            qc = g * 2 + i
                            for s in range(G6):
                                nc.tensor.transpose(
                                    ptq[:, s, i * P:(i + 1) * P],
                                    qnat[:, qc, s * P:(s + 1) * P], ident,
                                )
                        for s in range(G6):
                            nc.gpsimd.scalar_tensor_tensor(
                                out=qT_sb[:, s, g * 2 * P:(g + 1) * 2 * P],
                                in0=ptq[:, s], scalar=lnq_g_sb[:, s:s + 1],
                                in1=lnq_b_sb[:, s:s + 1].to_broadcast([P, 2 * P]),
                                op0=ALU.mult, op1=ALU.add,
                            )

            # k' projection + transposed copy-out with pos fold
            def kchunk(c, pend):
                py = ps_p.tile([P, D], f32, tag="py")
                proj_chunk(py, xvT_sb, wk_sb, D, c)
                mv = stats(py, statp)
                rstd = rstd_of(mv, statp, "kr")
                negmr = negmr_of(mv, rstd, statp, "kn")
                pair = c // 2
                if c % 2 == 0:
                    pend["t"] = knp.tile([P, 2, D], bf16, tag="knat",
                                         name=f"knat{c}")
                nc.scalar.activation(
                    out=pend["t"][:, c % 2], in_=py,
                    func=ACTF.Identity, bias=negmr, scale=rstd,
                )
                pend[pair] = pend["t"]

            def ktranspose(pair, pend):
                knat2 = pend.pop(pair)
                pos_sb = postp.tile([P, G6, 2 * P], bf16, tag="pos")
                nc.sync.dma_start(
                    out=pos_sb,
                    in_=posTb.rearrange("(s p) n -> p s n", p=P)[
                        :, :, pair * 2 * P:(pair + 1) * 2 * P],
                )
                pt = ps_t.tile([P, G6, 2 * P], bf16, tag="pt")
                for i in range(2):
                    for s in range(G6):
                        nc.tensor.transpose(
                            pt[:, s, i * P:(i + 1) * P],
                            knat2[:, i, s * P:(s + 1) * P], ident,
                        )
                for s in range(G6):
                    nc.gpsimd.scalar_tensor_tensor(
                        out=kT_sb[:, s, pair * 2 * P:(pair + 1) * 2 * P],
                        in0=pt[:, s], scalar=lnk_g_sb[:, s:s + 1],
                        in1=pos_sb[:, s], op0=ALU.mult, op1=ALU.add,
                    )

            pend = {}
            for c in range(MCH):
                kchunk(c, pend)
                if c >= 3 and c % 2 == 1:
                    ktranspose((c - 1) // 2 - 1, pend)  # one-pair lag
            ktranspose(MCH // 2 - 1, pend)

        # ---------------- attention: scores/exp (ACT-bound) + lagged av ----
        with (
            tc.tile_pool(name="atp", bufs=2) as atp,
            tc.tile_pool(name="rin", bufs=4) as rinp,
            tc.tile_pool(name="ps_s", bufs=2, space="PSUM") as ps_s,
            tc.tile_pool(name="ps_o", bufs=2, space="PSUM") as ps_o,
        ):
            def scores_group(at_h, h, mc0, w):
                p0 = DH * (h % 2)
                grp = h // 2
                ps = ps_s.tile([P, 3, S], f32, tag="ps3")
                for j in range(w):
                    mc = mc0 + j
                    nc.tensor.matmul(
                        ps[:, j],
                        kT_sb[p0:p0 + DH, grp, mc * P:(mc + 1) * P],
                        qT_sb[p0:p0 + DH, grp, :],
                        start=True, stop=True,
                    )
                nc.scalar.activation(
                    out=at_h[:, mc0:mc0 + w, :], in_=ps[:, :w],
                    func=ACTF.Exp, scale=SCALE,
                )

            def av_head(at_h, h):
                po = ps_o.tile([P, NCH, DH + 1], f32, tag="po")
                for qc in range(NCH):
                    for mc in range(MCH):
                        nc.tensor.matmul(
                            po[:, qc],
                            at_h[:, mc, qc * P:(qc + 1) * P],
                            vaug_sb[:, mc, h, :],
                            start=(mc == 0), stop=(mc == MCH - 1),
                        )
                    rinv = rinp.tile([P, 1], f32, tag="rin")
                    nc.vector.reciprocal(out=rinv, in_=po[:, qc, DH:DH + 1])
                    nc.gpsimd.tensor_scalar_mul(
                        out=onat_sb[:, qc, h], in0=po[:, qc, 0:DH],
                        scalar1=rinv,
                    )

            prev = None
            for h in range(H):
                at_h = atp.tile([P, MCH, S], bf16, tag="at", name=f"at{h}")
                for mc0, w in [(0, 3), (3, 3), (6, 3), (9, 3), (12, 3),
                               (15, 1)]:
                    scores_group(at_h, h, mc0, w)
                if prev is not None:
                    av_head(*prev)
                prev = (at_h, h)
            av_head(*prev)

        # transpose attention output for the output projection
        with tc.tile_pool(name="ps_t2", bufs=1, space="PSUM") as ps_t2:
            if True:
                for g in range(2):
                    pt = ps_t2.tile([P, G6, 2 * P], bf16, tag="pt2")
                    for i in range(2):
                        qc = g * 2 + i
                        src = onat_sb[:, qc].rearrange("p h d -> p (h d)")
                        for s in range(G6):
                            nc.tensor.transpose(
                                pt[:, s, i * P:(i + 1) * P],
                                src[:, s * P:(s + 1) * P], ident,
                            )
                    for s in range(G6):
                        nc.gpsimd.tensor_copy(
                            out=outT_sb[:, s, g * 2 * P:(g + 1) * 2 * P],
                            in_=pt[:, s],
                        )

        # ---------------- phase 3: out proj, gate, fuse, final LN ----------
        with (
            tc.tile_pool(name="ph3", bufs=1) as ph3,
            tc.tile_pool(name="fw", bufs=2) as fw,
            tc.tile_pool(name="st3", bufs=4) as st3,
            tc.tile_pool(name="ps_z", bufs=2, space="PSUM") as ps_z,
            tc.tile_pool(name="ps_t3", bufs=1, space="PSUM") as ps_t3,
        ):
            woT_sb = ph3.tile([P, G6, D], bf16)
            nc.sync.dma_start(out=woT_sb, in_=woT.rearrange("(s p) o -> p s o", p=P))
            vis_sb = ph3.tile([P, NCH, D], f32)
            nc.scalar.dma_start(out=vis_sb, in_=vis_nat.rearrange("(c p) o -> p c o", p=P))
            visT_sb = ph3.tile([P, G6, S], bf16)
            nc.scalar.dma_start(out=visT_sb, in_=visT_own.rearrange("(s p) n -> p s n", p=P))
            gwv_sb = ph3.tile([P, G6, D], bf16)
            nc.sync.dma_start(
                out=gwv_sb,
                in_=gwT.rearrange("(s p) o -> p s o", p=P)[0:P, 0:G6, :],
            )
            gwz_sb = ph3.tile([P, G6, D], bf16)
            nc.sync.dma_start(
                out=gwz_sb,
                in_=gwT.rearrange("(g s p) o -> p (g s) o", p=P, g=2)[:, G6:, :],
            )
            if has_gb:
                gb_sb = ph3.tile([1, D], bf16)
                nc.sync.dma_start(out=gb_sb, in_=gb.ap())
            if has_bo:
                bo_sb = ph3.tile([1, D], bf16)
                nc.sync.dma_start(out=bo_sb, in_=bo_a.ap())

            z_sb = ph3.tile([P, NCH, D], f32)
            zT_sb = ph3.tile([P, G6, S], bf16)
            gsig = ph3.tile([P, NCH, D], bf16)
            gbc = ph3.tile([P, D], f32)
            bbc = ph3.tile([P, D], f32)

            # broadcast final-LN gain/bias across partitions via K=1 matmul
            for dst, src_row in ((gbc, lnfw_sb), (bbc, lnfb_sb)):
                pb = ps_z.tile([P, D], f32, tag="pz")
                for o0, o1 in HALves:
                    nc.tensor.matmul(
                        pb[:, o0:o1], ones_f32, src_row[:, o0:o1],
                        start=True, stop=True,
                    )
                nc.vector.tensor_copy(out=dst, in_=pb)

            # z = attout @ woT' (+bo)
            for c in range(NCH):
                pz = ps_z.tile([P, D], f32, tag="pz")
                for o0, o1 in HALves:
                    for s in range(G6):
                        nc.tensor.matmul(
                            pz[:, o0:o1],
                            outT_sb[:, s, c * P:(c + 1) * P],
                            woT_sb[:, s, o0:o1],
                            start=(s == 0), stop=(not has_bo and s == G6 - 1),
                        )
                    if has_bo:
                        nc.tensor.matmul(
                            pz[:, o0:o1], ones_bf, bo_sb[:, o0:o1],
                            start=False, stop=True,
                        )
                nc.scalar.copy(out=z_sb[:, c], in_=pz)

            # zT for the gate matmul
            for g in range(2):
                pt = ps_t3.tile([P, G6, 2 * P], f32, tag="pt3")
                for i in range(2):
                    c = g * 2 + i
                    for s in range(G6):
                        nc.tensor.transpose(
                            pt[:, s, i * P:(i + 1) * P],
                            z_sb[:, c, s * P:(s + 1) * P], ident_f32,
                        )
                for s in range(G6):
                    nc.gpsimd.tensor_copy(
                        out=zT_sb[:, s, g * 2 * P:(g + 1) * 2 * P],
                        in_=pt[:, s],
                    )

            # gate = sigmoid([vis, z] @ gwT (+gb))
            for c in range(NCH):
                pg = ps_z.tile([P, D], f32, tag="pz")
                for o0, o1 in HALves:
                    for s in range(G6):
                        nc.tensor.matmul(
                            pg[:, o0:o1],
                            visT_sb[:, s, c * P:(c + 1) * P],
                            gwv_sb[:, s, o0:o1],
                            start=(s == 0), stop=False,
                        )
                    for s in range(G6):
                        nc.tensor.matmul(
                            pg[:, o0:o1],
                            zT_sb[:, s, c * P:(c + 1) * P],
                            gwz_sb[:, s, o0:o1],
                            start=False,
                            stop=(not has_gb and s == G6 - 1),
                        )
                    if has_gb:
                        nc.tensor.matmul(
                            pg[:, o0:o1], ones_bf, gb_sb[:, o0:o1],
                            start=False, stop=True,
                        )
                nc.scalar.activation(out=gsig[:, c], in_=pg, func=ACTF.Sigmoid)

            # fuse + final LN
            for c in range(NCH):
                dvz = fw.tile([P, D], f32, tag="dvz")
                nc.gpsimd.tensor_tensor(
                    out=dvz, in0=vis_sb[:, c], in1=z_sb[:, c], op=ALU.subtract,
                )
                fus = fw.tile([P, D], f32, tag="fus")
                nc.vector.tensor_tensor(out=fus, in0=gsig[:, c], in1=dvz,
                                        op=ALU.mult)
                nc.vector.tensor_tensor(out=fus, in0=fus, in1=z_sb[:, c],
                                        op=ALU.add)
                mv = stats(fus, st3)
                rstd = rstd_of(mv, st3, "fr")
                negmr = negmr_of(mv, rstd, st3, "fn")
                tnorm = fw.tile([P, D], f32, tag="tn")
                nc.scalar.activation(
                    out=tnorm, in_=fus, func=ACTF.Identity,
                    bias=negmr, scale=rstd,
                )
                nc.vector.tensor_tensor(out=tnorm, in0=tnorm, in1=gbc,
                                        op=ALU.mult)
                nc.gpsimd.tensor_tensor(out=tnorm, in0=tnorm, in1=bbc,
                                        op=ALU.add)
                nc.sync.dma_start(
                    out=out_rows.rearrange("(c p) o -> p c o", p=P)[:, c],
                    in_=tnorm,
                )

    nc.compile()
    return nc


def _prepare_in_maps(inputs):
    f32 = np.float32
    vis = np.asarray(inputs["visible_features"], f32)
    inf = np.asarray(inputs["infrared_features"], f32)
    wq = np.asarray(inputs["wq"], f32)
    bq = np.asarray(inputs["bq"], f32)
    lnq_w = np.asarray(inputs["lnq_w"], f32)
    lnq_b = np.asarray(inputs["lnq_b"], f32)
    wk = np.asarray(inputs["wk"], f32)
    bk = np.asarray(inputs["bk"], f32)
    lnk_w = np.asarray(inputs["lnk_w"], f32)
    lnk_b = np.asarray(inputs["lnk_b"], f32)
    wv = np.asarray(inputs["wv"], f32)
    bv = np.asarray(inputs["bv"], f32)
    lnv_w = np.asarray(inputs["lnv_w"], f32)
    lnv_b = np.asarray(inputs["lnv_b"], f32)
    pos = np.asarray(inputs["pos_emb"], f32)[:N]
    wo = np.asarray(inputs["wo"], f32)
    bo = np.asarray(inputs["bo"], f32)
    gw = np.asarray(inputs["gate_w"], f32)
    gb_ = np.asarray(inputs["gate_b"], f32)
    ln_w = np.asarray(inputs["ln_w"], f32)
    ln_b = np.asarray(inputs["ln_b"], f32)

    wqkvT = np.concatenate([wq.T, wk.T, wv.T], axis=1).astype(BF)
    bqkv = np.concatenate([bq, bk, bv])[None]
    woT = ((wo * lnv_w[None, :]).T).astype(BF)
    bo_a = (bo + wo @ lnv_b)[None]
    gwT = gw.T.astype(BF)
    lnq_g = np.ascontiguousarray(lnq_w.reshape(G6, P).T)
    lnq_b2 = np.ascontiguousarray(lnq_b.reshape(G6, P).T)
    lnk_g = np.ascontiguousarray(lnk_w.reshape(G6, P).T)
    lnf = np.stack([ln_w, ln_b])
    flags = (
        bool(np.any(bqkv)), bool(np.any(bo_a)), bool(np.any(gb_)),
    )

    in_maps = []
    for c in range(CORES):
        b, r0 = c // GROUP, (c % GROUP) * S
        m = {
            "xqT": np.ascontiguousarray(inf[b, r0:r0 + S].T).astype(BF),
            "xvTf": np.ascontiguousarray(vis[b].T).astype(BF),
            "visT_own": np.ascontiguousarray(vis[b, r0:r0 + S].T).astype(BF),
            "vis_nat": np.ascontiguousarray(vis[b, r0:r0 + S]),
            "posTb": np.ascontiguousarray(
                pos.T / SCALE + lnk_b[:, None]
            ).astype(BF),
            "wqkvT": np.ascontiguousarray(wqkvT),
            "woT": np.ascontiguousarray(woT),
            "gwT": np.ascontiguousarray(gwT),
            "lnq_g": lnq_g,
            "lnq_b": lnq_b2,
            "lnk_g": lnk_g,
            "lnf": lnf,
        }
        if flags[0]:
            m["bqkv"] = np.ascontiguousarray(bqkv).astype(BF)
        if flags[1]:
            m["bo_a"] = np.ascontiguousarray(bo_a).astype(BF)
        if flags[2]:
            m["gb"] = np.ascontiguousarray(gb_[None]).astype(BF)
        in_maps.append(m)
    return in_maps, flags


def kernel(trace=False, **inputs):
    from concourse.bass_utils import run_bass_kernel_spmd

    in_maps, flags = _prepare_in_maps(inputs)
    key = ("nc",) + flags
    if key not in _CACHE:
        _CACHE[key] = _build(*flags)
    nc = _CACHE[key]
    _CACHE["nc"] = nc
    res = run_bass_kernel_spmd(
        nc, in_maps, core_ids=list(range(CORES)), trace=trace
    )
    out = np.empty((B, N, D), np.float32)
    for c in range(CORES):
        b, r0 = c // GROUP, (c % GROUP) * S
        out[b, r0:r0 + S] = res.results[c]["out_rows"]
    _CACHE["last_result"] = res
    return out


# revision 20
# speedup vs baseline: 1.8831x; 1.0277x over previous
"""Trainium2 Bass kernel for nn_CustomCrossModalAttention (B=2, N=2048, D=768, H=12).

Sharding (8 cores, ZERO collectives):
  - core c owns batch b = c//4 and query rows [512*(c%4), 512*(c%4)+512).
  - k' and v are computed REDUNDANTLY for all 2048 keys of the core's batch
    (the cost-model prices AllGather at 15us + out_bytes/40GB/s, so the two
    baseline gathers cost 267us -- far more than the +46us of replicated PE
    matmul work).
  - Attention, output proj, gate, fuse, final LN are row-parallel on the
    core's own 512 query rows.

Algebra (all exact, matching the reference):
  - scores*scale + q@pos == scale * (q @ (k + pos/scale)^T); pos term and
    lnk bias folded into kT during the transpose copy-out.
  - LN_v gain/bias folded into wo / bo on the host.
  - rstd = exp(-0.5*ln(var+eps)) so the ACT engine never leaves the
    natural_log_exp table during phase1/attention (exp lives there too).

Engines: PE does all matmuls/transposes in bf16 (1 cyc/row); ACT does exp
(the 12.6M-element softmax exp is its ~95us floor) + q-applies; DVE does
bn_stats/aggr + k/v applies; Pool does transposed-copy-outs (STT with
gain*x+bias), v raw psum->sbuf copies, and the attention division.
"""

import numpy as np
import ml_dtypes

B, N, D = 2, 2048, 768
H, DH = 12, 64
P = 128
CORES, GROUP = 8, 4
S = 512            # query rows per core
NCH = S // P       # 4 own row chunks
MCH = N // P       # 16 key row chunks
G6 = D // P        # 6
SCALE = DH ** -0.5
EPS = 1e-5

BF = ml_dtypes.bfloat16

_CACHE = {}


def _build(has_bqkv, has_bo, has_gb):
    from contextlib import ExitStack

    import concourse.bacc as bacc
    import concourse.mybir as mybir
    import concourse.tile as tile
    from concourse.masks import make_identity

    f32 = mybir.dt.float32
    bf16 = mybir.dt.bfloat16
    ALU = mybir.AluOpType
    ACTF = mybir.ActivationFunctionType

    nc = bacc.Bacc("TRN2", target_bir_lowering=False, num_devices=CORES)

    def din(name, shape, dt=bf16):
        return nc.dram_tensor(name, shape, dt, kind="ExternalInput")

    xqT = din("xqT", [D, S])            # own infrared rows, transposed
    xvTf = din("xvTf", [D, N])          # FULL batch visible rows, transposed
    visT_own = din("visT_own", [D, S])  # own visible rows, transposed (gate)
    vis_nat = din("vis_nat", [S, D], f32)
    posTb = din("posTb", [D, N])        # (pos/scale + lnk_b), transposed
    wqkvT = din("wqkvT", [D, 3 * D])
    woT = din("woT", [D, D])            # (wo * lnv_w).T
    gwT = din("gwT", [2 * D, D])
    lnq_g = din("lnq_g", [P, G6], f32)
    lnq_b = din("lnq_b", [P, G6], f32)
    lnk_g = din("lnk_g", [P, G6], f32)
    lnf = din("lnf", [2, D], f32)
    bqkv = din("bqkv", [1, 3 * D]) if has_bqkv else None
    bo_a = din("bo_a", [1, D]) if has_bo else None
    gb = din("gb", [1, D]) if has_gb else None
    out_rows = nc.dram_tensor("out_rows", [S, D], f32, kind="ExternalOutput")

    POSC = 8                 # posTb streamed in 8 column chunks of 256
    PW = N // POSC           # 256

    with tile.TileContext(nc) as tc, ExitStack() as ctx:
        const = ctx.enter_context(tc.tile_pool(name="const", bufs=1))

        ident = const.tile([P, P], bf16)
        make_identity(nc, ident)
        ident_f32 = const.tile([P, P], f32)
        make_identity(nc, ident_f32)
        eps_t = const.tile([P, 1], f32)
        nc.vector.memset(eps_t, EPS)
        ones_bf = const.tile([1, P], bf16)
        nc.vector.memset(ones_bf, 1.0)
        ones_f32 = const.tile([1, P], f32)
        nc.vector.memset(ones_f32, 1.0)

        lnq_g_sb = const.tile([P, G6], f32)
        nc.scalar.dma_start(out=lnq_g_sb, in_=lnq_g.ap())
        lnq_b_sb = const.tile([P, G6], f32)
        nc.scalar.dma_start(out=lnq_b_sb, in_=lnq_b.ap())
        lnk_g_sb = const.tile([P, G6], f32)
        nc.scalar.dma_start(out=lnk_g_sb, in_=lnk_g.ap())
        lnfw_sb = const.tile([1, D], f32)
        nc.scalar.dma_start(out=lnfw_sb, in_=lnf.ap()[0:1, :])
        lnfb_sb = const.tile([1, D], f32)
        nc.scalar.dma_start(out=lnfb_sb, in_=lnf.ap()[1:2, :])

        # long-lived activation tensors
        qT_sb = const.tile([P, G6, S], bf16)
        kT_sb = const.tile([P, G6, N], bf16)
        outT_sb = const.tile([P, G6, S], bf16)

        if has_bqkv:
            bqkv_sb = const.tile([1, 3 * D], bf16)
            nc.scalar.dma_start(out=bqkv_sb, in_=bqkv.ap())

        HALves = [(0, 512), (512, D)]

        def proj_chunk(py, lhsT_sb, w_sb, w_off, c):
            """py[128,768] = x-chunk-c @ W (+bias via ones row when present)."""
            for o0, o1 in HALves:
                for s in range(G6):
                    nc.tensor.matmul(
                        py[:, o0:o1],
                        lhsT_sb[:, s, c * P:(c + 1) * P],
                        w_sb[:, s, o0:o1],
                        start=(s == 0), stop=(not has_bqkv and s == G6 - 1),
                    )
                if has_bqkv:
                    nc.tensor.matmul(
                        py[:, o0:o1], ones_bf,
                        bqkv_sb[:, w_off + o0:w_off + o1],
                        start=False, stop=True,
                    )

        def stats(py, pool):
            """DVE bn_stats -> (mean, var) [P,2]."""
            st = pool.tile([P, 2, 6], f32, tag="st")
            for i in range(2):
                nc.vector.bn_stats(out=st[:, i], in_=py[:, i * 384:(i + 1) * 384])
            mv = pool.tile([P, 2], f32, tag="mv")
            nc.vector.bn_aggr(out=mv, in_=st)
            return mv

        def rstd_of(mv, pool, tag="rs"):
            """rstd = 1/sqrt(var+eps): ACT Sqrt then DVE reciprocal."""
            rstd = pool.tile([P, 1], f32, tag=tag + "r")
            nc.scalar.activation(out=rstd, in_=mv[:, 1:2], func=ACTF.Sqrt,
                                 bias=eps_t, scale=1.0)
            nc.vector.reciprocal(out=rstd, in_=rstd)
            return rstd

        def negmr_of(mv, rstd, pool, tag="nm"):
            negmr = pool.tile([P, 1], f32, tag=tag)
            nc.vector.tensor_scalar(
                out=negmr, in0=mv[:, 0:1], scalar1=rstd, scalar2=-1.0,
                op0=ALU.mult, op1=ALU.mult,
            )
            return negmr

        mid = ctx.enter_context(tc.tile_pool(name="mid", bufs=1))
        xvT_sb = mid.tile([P, G6, N], bf16)
        wv_sb = mid.tile([P, G6, D], bf16)
        woT_sb = mid.tile([P, G6, D], bf16)
        vis_sb = mid.tile([P, NCH, D], f32)
        visT_sb = mid.tile([P, G6, S], bf16)
        gwv_sb = mid.tile([P, G6, D], bf16)
        gwz_sb = mid.tile([P, G6, D], bf16)
        nc.scalar.dma_start(out=woT_sb, in_=woT.rearrange("(s p) o -> p s o", p=P))
        nc.scalar.dma_start(out=vis_sb, in_=vis_nat.rearrange("(c p) o -> p c o", p=P))
        nc.scalar.dma_start(out=visT_sb, in_=visT_own.rearrange("(s p) n -> p s n", p=P))
        nc.scalar.dma_start(
            out=gwv_sb,
            in_=gwT.rearrange("(s p) o -> p s o", p=P)[0:P, 0:G6, :],
        )
        nc.scalar.dma_start(
            out=gwz_sb,
            in_=gwT.rearrange("(g s p) o -> p (g s) o", p=P, g=2)[:, G6:, :],
        )
        vaug_sb = mid.tile([P, MCH, H, DH + 1], bf16)
        nc.vector.memset(vaug_sb[:, :, :, DH:DH + 1], 1.0)
        onat_sb = mid.tile([P, NCH, H, DH], bf16)

        # ---------------- phase 1: q, v, k' (all LN sqrt before any exp) --
        with (
            tc.tile_pool(name="ph1", bufs=1) as ph1,
            tc.tile_pool(name="knp", bufs=3) as knp,
            tc.tile_pool(name="post", bufs=2) as postp,
            tc.tile_pool(name="stat", bufs=4) as statp,
            tc.tile_pool(name="ps_p", bufs=3, space="PSUM") as ps_p,
            tc.tile_pool(name="ps_t", bufs=1, space="PSUM") as ps_t,
        ):
            xqT_sb = ph1.tile([P, G6, S], bf16)
            nc.sync.dma_start(out=xqT_sb, in_=xqT.rearrange("(s p) n -> p s n", p=P))
            nc.gpsimd.dma_start(out=xvT_sb, in_=xvTf.rearrange("(s p) n -> p s n", p=P))
            wq_sb = ph1.tile([P, G6, D], bf16)
            for h0, h1 in ((0, 512), (512, D)):
                nc.sync.dma_start(
                    out=wq_sb[:, :, h0:h1],
                    in_=wqkvT.rearrange("(s p) o -> p s o", p=P)[:, :, h0:h1],
                )
            nc.sync.dma_start(
                out=wv_sb,
                in_=wqkvT.rearrange("(s p) o -> p s o", p=P)[:, :, 2 * D:3 * D],
            )
            wk_sb = ph1.tile([P, G6, D], bf16)
            nc.sync.dma_start(
                out=wk_sb,
                in_=wqkvT.rearrange("(s p) o -> p s o", p=P)[:, :, D:2 * D],
            )
            qnat = ph1.tile([P, NCH, D], bf16)

            # q projection (apply on ACT)
            for c in range(NCH):
                py = ps_p.tile([P, D], f32, tag="py")
                proj_chunk(py, xqT_sb, wq_sb, 0, c)
                mv = stats(py, statp)
                rstd = rstd_of(mv, statp, "qr")
                negmr = negmr_of(mv, rstd, statp, "qn")
                nc.scalar.activation(
                    out=qnat[:, c], in_=py, func=ACTF.Identity,
                    bias=negmr, scale=rstd,
                )

            # v projection: normalize straight from psum into vaug (DVE)
            for c in range(MCH):
                pv = ps_p.tile([P, D], f32, tag="py")
                proj_chunk(pv, xvT_sb, wv_sb, 2 * D, c)
                mv = stats(pv, statp)
                rstd = rstd_of(mv, statp, "vr")
                negmr = negmr_of(mv, rstd, statp, "vn")
                nc.scalar.activation(
                    out=vaug_sb[:, c, :, 0:DH],
                    in_=pv.rearrange("p (h d) -> p h d", h=H),
                    func=ACTF.Identity, bias=negmr, scale=rstd,
                )
                if c == 1:
                    # q transpose + STT while v streams (qnat ready)
                    for g in range(2):
                        ptq = ps_t.tile([P, G6, 2 * P], bf16, tag="pt")
                        for i in range(2):
                            qc = g * 2 + i
                            for s in range(G6):
                                nc.tensor.transpose(
                                    ptq[:, s, i * P:(i + 1) * P],
                                    qnat[:, qc, s * P:(s + 1) * P], ident,
                                )
                        for s in range(G6):
                            nc.gpsimd.scalar_tensor_tensor(
                                out=qT_sb[:, s, g * 2 * P:(g + 1) * 2 * P],
                                in0=ptq[:, s], scalar=lnq_g_sb[:, s:s + 1],
                                in1=lnq_b_sb[:, s:s + 1].to_broadcast([P, 2 * P]),
                                op0=ALU.mult, op1=ALU.add,
                            )

            # k' projection + transposed copy-out with pos fold
            def kchunk(c, pend):
                py = ps_p.tile([P, D], f32, tag="py")
                proj_chunk(py, xvT_sb, wk_sb, D, c)
                mv = stats(py, statp)
                rstd = rstd_of(mv, statp, "kr")
                negmr = negmr_of(mv, rstd, statp, "kn")
                pair = c // 2
                if c % 2 == 0:
                    pend["t"] = knp.tile([P, 2, D], bf16, tag="knat",
                                         name=f"knat{c}")
                nc.scalar.activation(
                    out=pend["t"][:, c % 2], in_=py,
                    func=ACTF.Identity, bias=negmr, scale=rstd,
                )
                pend[pair] = pend["t"]

            def ktranspose(pair, pend):
                knat2 = pend.pop(pair)
                pos_sb = postp.tile([P, G6, 2 * P], bf16, tag="pos")
                nc.sync.dma_start(
                    out=pos_sb,
                    in_=posTb.rearrange("(s p) n -> p s n", p=P)[
                        :, :, pair * 2 * P:(pair + 1) * 2 * P],
                )
                pt = ps_t.tile([P, G6, 2 * P], bf16, tag="pt")
                for i in range(2):
                    for s in range(G6):
                        nc.tensor.transpose(
                            pt[:, s, i * P:(i + 1) * P],
                            knat2[:, i, s * P:(s + 1) * P], ident,
                        )
                for s in range(G6):
                    nc.gpsimd.scalar_tensor_tensor(
                        out=kT_sb[:, s, pair * 2 * P:(pair + 1) * 2 * P],
                        in0=pt[:, s], scalar=lnk_g_sb[:, s:s + 1],
                        in1=pos_sb[:, s], op0=ALU.mult, op1=ALU.add,
                    )

            pend = {}
            for c in range(MCH):
                kchunk(c, pend)
                if c >= 3 and c % 2 == 1:
                    ktranspose((c - 1) // 2 - 1, pend)  # one-pair lag
            ktranspose(MCH // 2 - 1, pend)

        # ---------------- attention: scores/exp (ACT-bound) + lagged av ----
        with (
            tc.tile_pool(name="atp", bufs=2) as atp,
            tc.tile_pool(name="rin", bufs=4) as rinp,
            tc.tile_pool(name="ps_s", bufs=2, space="PSUM") as ps_s,
            tc.tile_pool(name="ps_o", bufs=2, space="PSUM") as ps_o,
        ):
            def scores_group(at_h, h, mc0, w):
                p0 = DH * (h % 2)
                grp = h // 2
                ps = ps_s.tile([P, 3, S], f32, tag="ps3")
                for j in range(w):
                    mc = mc0 + j
                    nc.tensor.matmul(
                        ps[:, j],
                        kT_sb[p0:p0 + DH, grp, mc * P:(mc + 1) * P],
                        qT_sb[p0:p0 + DH, grp, :],
                        start=True, stop=True,
                    )
                nc.scalar.activation(
                    out=at_h[:, mc0:mc0 + w, :], in_=ps[:, :w],
                    func=ACTF.Exp, scale=SCALE,
                )

            def av_head(at_h, h):
                po = ps_o.tile([P, NCH, DH + 1], f32, tag="po")
                for qc in range(NCH):
                    for mc in range(MCH):
                        nc.tensor.matmul(
                            po[:, qc],
                            at_h[:, mc, qc * P:(qc + 1) * P],
                            vaug_sb[:, mc, h, :],
                            start=(mc == 0), stop=(mc == MCH - 1),
                        )
                    rinv = rinp.tile([P, 1], f32, tag="rin")
                    nc.vector.reciprocal(out=rinv, in_=po[:, qc, DH:DH + 1])
                    nc.gpsimd.tensor_scalar_mul(
                        out=onat_sb[:, qc, h], in0=po[:, qc, 0:DH],
                        scalar1=rinv,
                    )

            prev = None
            for h in range(H):
                at_h = atp.tile([P, MCH, S], bf16, tag="at", name=f"at{h}")
                for mc0, w in [(0, 3), (3, 3), (6, 3), (9, 3), (12, 3),
                               (15, 1)]:
                    scores_group(at_h, h, mc0, w)
                if prev is not None:
                    av_head(*prev)
                prev = (at_h, h)
            av_head(*prev)

        # transpose attention output for the output projection
        with tc.tile_pool(name="ps_t2", bufs=1, space="PSUM") as ps_t2:
            if True:
                for g in range(2):
                    pt = ps_t2.tile([P, G6, 2 * P], bf16, tag="pt2")
                    for i in range(2):
                        qc = g * 2 + i
                        src = onat_sb[:, qc].rearrange("p h d -> p (h d)")
                        for s in range(G6):
                            nc.tensor.transpose(
                                pt[:, s, i * P:(i + 1) * P],
                                src[:, s * P:(s + 1) * P], ident,
                            )
                    for s in range(G6):
                        nc.gpsimd.tensor_copy(
                            out=outT_sb[:, s, g * 2 * P:(g + 1) * 2 * P],
                            in_=pt[:, s],
                        )

        # ---------------- phase 3: out proj, gate, fuse, final LN ----------
        with (
            tc.tile_pool(name="ph3", bufs=1) as ph3,
            tc.tile_pool(name="fw", bufs=2) as fw,
            tc.tile_pool(name="st3", bufs=4) as st3,
            tc.tile_pool(name="ps_z", bufs=2, space="PSUM") as ps_z,
            tc.tile_pool(name="ps_t3", bufs=1, space="PSUM") as ps_t3,
        ):
            if has_gb:
                gb_sb = ph3.tile([1, D], bf16)
                nc.sync.dma_start(out=gb_sb, in_=gb.ap())
            if has_bo:
                bo_sb = ph3.tile([1, D], bf16)
                nc.sync.dma_start(out=bo_sb, in_=bo_a.ap())

            z_sb = ph3.tile([P, NCH, D], f32)
            zT_sb = ph3.tile([P, G6, S], bf16)
            gsig = ph3.tile([P, NCH, D], bf16)
            gbc = ph3.tile([P, D], f32)
            bbc = ph3.tile([P, D], f32)

            # broadcast final-LN gain/bias across partitions via K=1 matmul
            for dst, src_row in ((gbc, lnfw_sb), (bbc, lnfb_sb)):
                pb = ps_z.tile([P, D], f32, tag="pz")
                for o0, o1 in HALves:
                    nc.tensor.matmul(
                        pb[:, o0:o1], ones_f32, src_row[:, o0:o1],
                        start=True, stop=True,
                    )
                nc.vector.tensor_copy(out=dst, in_=pb)

            # z = attout @ woT' (+bo)
            for c in range(NCH):
                pz = ps_z.tile([P, D], f32, tag="pz")
                for o0, o1 in HALves:
                    for s in range(G6):
                        nc.tensor.matmul(
                            pz[:, o0:o1],
                            outT_sb[:, s, c * P:(c + 1) * P],
                            woT_sb[:, s, o0:o1],
                            start=(s == 0), stop=(not has_bo and s == G6 - 1),
                        )
                    if has_bo:
                        nc.tensor.matmul(
                            pz[:, o0:o1], ones_bf, bo_sb[:, o0:o1],
                            start=False, stop=True,
                        )
                nc.scalar.copy(out=z_sb[:, c], in_=pz)

            # zT for the gate matmul
            for g in range(2):
                pt = ps_t3.tile([P, G6, 2 * P], f32, tag="pt3")
                for i in range(2):
                    c = g * 2 + i
                    for s in range(G6):
                        nc.tensor.transpose(
                            pt[:, s, i * P:(i + 1) * P],
                            z_sb[:, c, s * P:(s + 1) * P], ident_f32,
                        )
                for s in range(G6):
                    nc.gpsimd.tensor_copy(
                        out=zT_sb[:, s, g * 2 * P:(g + 1) * 2 * P],
                        in_=pt[:, s],
                    )

            # gate = sigmoid([vis, z] @ gwT (+gb))
            for c in range(NCH):
                pg = ps_z.tile([P, D], f32, tag="pz")
                for o0, o1 in HALves:
                    for s in range(G6):
                        nc.tensor.matmul(
                            pg[:, o0:o1],
                            visT_sb[:, s, c * P:(c + 1) * P],
                            gwv_sb[:, s, o0:o1],
                            start=(s == 0), stop=False,
                        )
                    for s in range(G6):
                        nc.tensor.matmul(
                            pg[:, o0:o1],
                            zT_sb[:, s, c * P:(c + 1) * P],
                            gwz_sb[:, s, o0:o1],
                            start=False,
                            stop=(not has_gb and s == G6 - 1),
                        )
                    if has_gb:
                        nc.tensor.matmul(
                            pg[:, o0:o1], ones_bf, gb_sb[:, o0:o1],
                            start=False, stop=True,
                        )
                nc.scalar.activation(out=gsig[:, c], in_=pg, func=ACTF.Sigmoid)

            # fuse + final LN
            for c in range(NCH):
                dvz = fw.tile([P, D], f32, tag="dvz")
                nc.gpsimd.tensor_tensor(
                    out=dvz, in0=vis_sb[:, c], in1=z_sb[:, c], op=ALU.subtract,
                )
                fus = fw.tile([P, D], f32, tag="fus")
                nc.vector.tensor_tensor(out=fus, in0=gsig[:, c], in1=dvz,
                                        op=ALU.mult)
                nc.vector.tensor_tensor(out=fus, in0=fus, in1=z_sb[:, c],
                                        op=ALU.add)
                mv = stats(fus, st3)
                rstd = rstd_of(mv, st3, "fr")
                negmr = negmr_of(mv, rstd, st3, "fn")
                tnorm = fw.tile([P, D], f32, tag="tn")
                nc.scalar.activation(
                    out=tnorm, in_=fus, func=ACTF.Identity,
                    bias=negmr, scale=rstd,
                )
                nc.vector.tensor_tensor(out=tnorm, in0=tnorm, in1=gbc,
                                        op=ALU.mult)
                nc.gpsimd.tensor_tensor(out=tnorm, in0=tnorm, in1=bbc,
                                        op=ALU.add)
                nc.sync.dma_start(
                    out=out_rows.rearrange("(c p) o -> p c o", p=P)[:, c],
                    in_=tnorm,
                )

    nc.compile()
    return nc


def _prepare_in_maps(inputs):
    f32 = np.float32
    vis = np.asarray(inputs["visible_features"], f32)
    inf = np.asarray(inputs["infrared_features"], f32)
    wq = np.asarray(inputs["wq"], f32)
    bq = np.asarray(inputs["bq"], f32)
    lnq_w = np.asarray(inputs["lnq_w"], f32)
    lnq_b = np.asarray(inputs["lnq_b"], f32)
    wk = np.asarray(inputs["wk"], f32)
    bk = np.asarray(inputs["bk"], f32)
    lnk_w = np.asarray(inputs["lnk_w"], f32)
    lnk_b = np.asarray(inputs["lnk_b"], f32)
    wv = np.asarray(inputs["wv"], f32)
    bv = np.asarray(inputs["bv"], f32)
    lnv_w = np.asarray(inputs["lnv_w"], f32)
    lnv_b = np.asarray(inputs["lnv_b"], f32)
    pos = np.asarray(inputs["pos_emb"], f32)[:N]
    wo = np.asarray(inputs["wo"], f32)
    bo = np.asarray(inputs["bo"], f32)
    gw = np.asarray(inputs["gate_w"], f32)
    gb_ = np.asarray(inputs["gate_b"], f32)
    ln_w = np.asarray(inputs["ln_w"], f32)
    ln_b = np.asarray(inputs["ln_b"], f32)

    wqkvT = np.concatenate([wq.T, wk.T, wv.T], axis=1).astype(BF)
    bqkv = np.concatenate([bq, bk, bv])[None]
    woT = ((wo * lnv_w[None, :]).T).astype(BF)
    bo_a = (bo + wo @ lnv_b)[None]
    gwT = gw.T.astype(BF)
    lnq_g = np.ascontiguousarray(lnq_w.reshape(G6, P).T)
    lnq_b2 = np.ascontiguousarray(lnq_b.reshape(G6, P).T)
    lnk_g = np.ascontiguousarray(lnk_w.reshape(G6, P).T)
    lnf = np.stack([ln_w, ln_b])
    flags = (
        bool(np.any(bqkv)), bool(np.any(bo_a)), bool(np.any(gb_)),
    )

    in_maps = []
    for c in range(CORES):
        b, r0 = c // GROUP, (c % GROUP) * S
        m = {
            "xqT": np.ascontiguousarray(inf[b, r0:r0 + S].T).astype(BF),
            "xvTf": np.ascontiguousarray(vis[b].T).astype(BF),
            "visT_own": np.ascontiguousarray(vis[b, r0:r0 + S].T).astype(BF),
            "vis_nat": np.ascontiguousarray(vis[b, r0:r0 + S]),
            "posTb": np.ascontiguousarray(
                pos.T / SCALE + lnk_b[:, None]
            ).astype(BF),
            "wqkvT": np.ascontiguousarray(wqkvT),
            "woT": np.ascontiguousarray(woT),
            "gwT": np.ascontiguousarray(gwT),
            "lnq_g": lnq_g,
            "lnq_b": lnq_b2,
            "lnk_g": lnk_g,
            "lnf": lnf,
        }
        if flags[0]:
            m["bqkv"] = np.ascontiguousarray(bqkv).astype(BF)
        if flags[1]:
            m["bo_a"] = np.ascontiguousarray(bo_a).astype(BF)
        if flags[2]:
            m["gb"] = np.ascontiguousarray(gb_[None]).astype(BF)
        in_maps.append(m)
    return in_maps, flags


def kernel(trace=False, **inputs):
    from concourse.bass_utils import run_bass_kernel_spmd

    in_maps, flags = _prepare_in_maps(inputs)
    key = ("nc",) + flags
    if key not in _CACHE:
        _CACHE[key] = _build(*flags)
    nc = _CACHE[key]
    _CACHE["nc"] = nc
    res = run_bass_kernel_spmd(
        nc, in_maps, core_ids=list(range(CORES)), trace=trace
    )
    out = np.empty((B, N, D), np.float32)
    for c in range(CORES):
        b, r0 = c // GROUP, (c % GROUP) * S
        out[b, r0:r0 + S] = res.results[c]["out_rows"]
    _CACHE["last_result"] = res
    return out


# revision 21
# speedup vs baseline: 1.8913x; 1.0044x over previous
"""Trainium2 Bass kernel for nn_CustomCrossModalAttention (B=2, N=2048, D=768, H=12).

Sharding (8 cores, ZERO collectives):
  - core c owns batch b = c//4 and query rows [512*(c%4), 512*(c%4)+512).
  - k' and v are computed REDUNDANTLY for all 2048 keys of the core's batch
    (the cost-model prices AllGather at 15us + out_bytes/40GB/s, so the two
    baseline gathers cost 267us -- far more than the +46us of replicated PE
    matmul work).
  - Attention, output proj, gate, fuse, final LN are row-parallel on the
    core's own 512 query rows.

Algebra (all exact, matching the reference):
  - scores*scale + q@pos == scale * (q @ (k + pos/scale)^T); pos term and
    lnk bias folded into kT during the transpose copy-out.
  - LN_v gain/bias folded into wo / bo on the host.
  - rstd = exp(-0.5*ln(var+eps)) so the ACT engine never leaves the
    natural_log_exp table during phase1/attention (exp lives there too).

Engines: PE does all matmuls/transposes in bf16 (1 cyc/row); ACT does exp
(the 12.6M-element softmax exp is its ~95us floor) + q-applies; DVE does
bn_stats/aggr + k/v applies; Pool does transposed-copy-outs (STT with
gain*x+bias), v raw psum->sbuf copies, and the attention division.
"""

import numpy as np
import ml_dtypes

B, N, D = 2, 2048, 768
H, DH = 12, 64
P = 128
CORES, GROUP = 8, 4
S = 512            # query rows per core
NCH = S // P       # 4 own row chunks
MCH = N // P       # 16 key row chunks
G6 = D // P        # 6
SCALE = DH ** -0.5
EPS = 1e-5

BF = ml_dtypes.bfloat16

_CACHE = {}


def _build(has_bqkv, has_bo, has_gb):
    from contextlib import ExitStack

    import concourse.bacc as bacc
    import concourse.mybir as mybir
    import concourse.tile as tile
    from concourse.masks import make_identity

    f32 = mybir.dt.float32
    bf16 = mybir.dt.bfloat16
    ALU = mybir.AluOpType
    ACTF = mybir.ActivationFunctionType

    nc = bacc.Bacc("TRN2", target_bir_lowering=False, num_devices=CORES)

    def din(name, shape, dt=bf16):
        return nc.dram_tensor(name, shape, dt, kind="ExternalInput")

    xqT = din("xqT", [D, S])            # own infrared rows, transposed
    xvTf = din("xvTf", [D, N])          # FULL batch visible rows, transposed
    visT_own = din("visT_own", [D, S])  # own visible rows, transposed (gate)
    vis_nat = din("vis_nat", [S, D], f32)
    posTb = din("posTb", [D, N])        # (pos/scale + lnk_b), transposed
    wqkvT = din("wqkvT", [D, 3 * D])
    woT = din("woT", [D, D])            # (wo * lnv_w).T
    gwT = din("gwT", [2 * D, D])
    lnq_g = din("lnq_g", [P, G6], f32)
    lnq_b = din("lnq_b", [P, G6], f32)
    lnk_g = din("lnk_g", [P, G6], f32)
    lnf = din("lnf", [2, D], f32)
    bqkv = din("bqkv", [1, 3 * D]) if has_bqkv else None
    bo_a = din("bo_a", [1, D]) if has_bo else None
    gb = din("gb", [1, D]) if has_gb else None
    out_rows = nc.dram_tensor("out_rows", [S, D], f32, kind="ExternalOutput")

    POSC = 8                 # posTb streamed in 8 column chunks of 256
    PW = N // POSC           # 256

    with tile.TileContext(nc) as tc, ExitStack() as ctx:
        const = ctx.enter_context(tc.tile_pool(name="const", bufs=1))

        ident = const.tile([P, P], bf16)
        make_identity(nc, ident)
        ident_f32 = const.tile([P, P], f32)
        make_identity(nc, ident_f32)
        eps_t = const.tile([P, 1], f32)
        nc.vector.memset(eps_t, EPS)
        ones_bf = const.tile([1, P], bf16)
        nc.vector.memset(ones_bf, 1.0)
        ones_f32 = const.tile([1, P], f32)
        nc.vector.memset(ones_f32, 1.0)

        lnq_g_sb = const.tile([P, G6], f32)
        nc.scalar.dma_start(out=lnq_g_sb, in_=lnq_g.ap())
        lnq_b_sb = const.tile([P, G6], f32)
        nc.scalar.dma_start(out=lnq_b_sb, in_=lnq_b.ap())
        lnk_g_sb = const.tile([P, G6], f32)
        nc.scalar.dma_start(out=lnk_g_sb, in_=lnk_g.ap())
        lnfw_sb = const.tile([1, D], f32)
        nc.scalar.dma_start(out=lnfw_sb, in_=lnf.ap()[0:1, :])
        lnfb_sb = const.tile([1, D], f32)
        nc.scalar.dma_start(out=lnfb_sb, in_=lnf.ap()[1:2, :])

        # long-lived activation tensors
        qT_sb = const.tile([P, G6, S], bf16)
        kT_sb = const.tile([P, G6, N], bf16)
        outT_sb = const.tile([P, G6, S], bf16)

        if has_bqkv:
            bqkv_sb = const.tile([1, 3 * D], bf16)
            nc.scalar.dma_start(out=bqkv_sb, in_=bqkv.ap())

        HALves = [(0, 512), (512, D)]

        def proj_chunk(py, lhsT_sb, w_sb, w_off, c):
            """py[128,768] = x-chunk-c @ W (+bias via ones row when present)."""
            for o0, o1 in HALves:
                for s in range(G6):
                    nc.tensor.matmul(
                        py[:, o0:o1],
                        lhsT_sb[:, s, c * P:(c + 1) * P],
                        w_sb[:, s, o0:o1],
                        start=(s == 0), stop=(not has_bqkv and s == G6 - 1),
                    )
                if has_bqkv:
                    nc.tensor.matmul(
                        py[:, o0:o1], ones_bf,
                        bqkv_sb[:, w_off + o0:w_off + o1],
                        start=False, stop=True,
                    )

        def stats(py, pool):
            """DVE bn_stats -> (mean, var) [P,2]."""
            st = pool.tile([P, 2, 6], f32, tag="st")
            for i in range(2):
                nc.vector.bn_stats(out=st[:, i], in_=py[:, i * 384:(i + 1) * 384])
            mv = pool.tile([P, 2], f32, tag="mv")
            nc.vector.bn_aggr(out=mv, in_=st)
            return mv

        def rstd_of(mv, pool, tag="rs"):
            """rstd = 1/sqrt(var+eps): ACT Sqrt then DVE reciprocal."""
            rstd = pool.tile([P, 1], f32, tag=tag + "r")
            nc.scalar.activation(out=rstd, in_=mv[:, 1:2], func=ACTF.Sqrt,
                                 bias=eps_t, scale=1.0)
            nc.vector.reciprocal(out=rstd, in_=rstd)
            return rstd

        def negmr_of(mv, rstd, pool, tag="nm"):
            negmr = pool.tile([P, 1], f32, tag=tag)
            nc.vector.tensor_scalar(
                out=negmr, in0=mv[:, 0:1], scalar1=rstd, scalar2=-1.0,
                op0=ALU.mult, op1=ALU.mult,
            )
            return negmr

        mid = ctx.enter_context(tc.tile_pool(name="mid", bufs=1))
        xvT_sb = mid.tile([P, G6, N], bf16)
        wv_sb = mid.tile([P, G6, D], bf16)
        woT_sb = mid.tile([P, G6, D], bf16)
        vis_sb = mid.tile([P, NCH, D], f32)
        visT_sb = mid.tile([P, G6, S], bf16)
        gwv_sb = mid.tile([P, G6, D], bf16)
        gwz_sb = mid.tile([P, G6, D], bf16)
        vaug_sb = mid.tile([P, MCH, H, DH + 1], bf16)
        nc.vector.memset(vaug_sb[:, :, :, DH:DH + 1], 1.0)
        onat_sb = mid.tile([P, NCH, H, DH], bf16)

        # ---------------- phase 1: q, v, k' (all LN sqrt before any exp) --
        with (
            tc.tile_pool(name="ph1", bufs=1) as ph1,
            tc.tile_pool(name="knp", bufs=3) as knp,
            tc.tile_pool(name="post", bufs=2) as postp,
            tc.tile_pool(name="stat", bufs=4) as statp,
            tc.tile_pool(name="ps_p", bufs=3, space="PSUM") as ps_p,
            tc.tile_pool(name="ps_t", bufs=1, space="PSUM") as ps_t,
        ):
            xqT_sb = ph1.tile([P, G6, S], bf16)
            nc.sync.dma_start(out=xqT_sb, in_=xqT.rearrange("(s p) n -> p s n", p=P))
            nc.gpsimd.dma_start(out=xvT_sb, in_=xvTf.rearrange("(s p) n -> p s n", p=P))
            wq_sb = ph1.tile([P, G6, D], bf16)
            for h0, h1 in ((0, 512), (512, D)):
                nc.sync.dma_start(
                    out=wq_sb[:, :, h0:h1],
                    in_=wqkvT.rearrange("(s p) o -> p s o", p=P)[:, :, h0:h1],
                )
            nc.sync.dma_start(
                out=wv_sb,
                in_=wqkvT.rearrange("(s p) o -> p s o", p=P)[:, :, 2 * D:3 * D],
            )
            wk_sb = ph1.tile([P, G6, D], bf16)
            nc.sync.dma_start(
                out=wk_sb,
                in_=wqkvT.rearrange("(s p) o -> p s o", p=P)[:, :, D:2 * D],
            )
            qnat = ph1.tile([P, NCH, D], bf16)

            # q projection (apply on ACT)
            for c in range(NCH):
                py = ps_p.tile([P, D], f32, tag="py")
                proj_chunk(py, xqT_sb, wq_sb, 0, c)
                mv = stats(py, statp)
                rstd = rstd_of(mv, statp, "qr")
                negmr = negmr_of(mv, rstd, statp, "qn")
                nc.scalar.activation(
                    out=qnat[:, c], in_=py, func=ACTF.Identity,
                    bias=negmr, scale=rstd,
                )

            # late prefetch of phase-3 weights (DMA idle from here on)
            nc.scalar.dma_start(out=woT_sb, in_=woT.rearrange("(s p) o -> p s o", p=P))
            nc.scalar.dma_start(out=vis_sb, in_=vis_nat.rearrange("(c p) o -> p c o", p=P))
            nc.scalar.dma_start(out=visT_sb, in_=visT_own.rearrange("(s p) n -> p s n", p=P))
            nc.scalar.dma_start(
                out=gwv_sb,
                in_=gwT.rearrange("(s p) o -> p s o", p=P)[0:P, 0:G6, :],
            )
            nc.scalar.dma_start(
                out=gwz_sb,
                in_=gwT.rearrange("(g s p) o -> p (g s) o", p=P, g=2)[:, G6:, :],
            )

            # v projection: normalize straight from psum into vaug (DVE)
            for c in range(MCH):
                pv = ps_p.tile([P, D], f32, tag="py")
                proj_chunk(pv, xvT_sb, wv_sb, 2 * D, c)
                mv = stats(pv, statp)
                rstd = rstd_of(mv, statp, "vr")
                negmr = negmr_of(mv, rstd, statp, "vn")
                nc.scalar.activation(
                    out=vaug_sb[:, c, :, 0:DH],
                    in_=pv.rearrange("p (h d) -> p h d", h=H),
                    func=ACTF.Identity, bias=negmr, scale=rstd,
                )
                if c == 1:
                    # q transpose + STT while v streams (qnat ready)
                    for g in range(2):
                        ptq = ps_t.tile([P, G6, 2 * P], bf16, tag="pt")
                        for i in range(2):
                            qc = g * 2 + i
                            for s in range(G6):
                                nc.tensor.transpose(
                                    ptq[:, s, i * P:(i + 1) * P],
                                    qnat[:, qc, s * P:(s + 1) * P], ident,
                                )
                        for s in range(G6):
                            nc.gpsimd.scalar_tensor_tensor(
                                out=qT_sb[:, s, g * 2 * P:(g + 1) * 2 * P],
                                in0=ptq[:, s], scalar=lnq_g_sb[:, s:s + 1],
                                in1=lnq_b_sb[:, s:s + 1].to_broadcast([P, 2 * P]),
                                op0=ALU.mult, op1=ALU.add,
                            )

            # k' projection + transposed copy-out with pos fold
            def kchunk(c, pend):
                py = ps_p.tile([P, D], f32, tag="py")
                proj_chunk(py, xvT_sb, wk_sb, D, c)
                mv = stats(py, statp)
                rstd = rstd_of(mv, statp, "kr")
                negmr = negmr_of(mv, rstd, statp, "kn")
                pair = c // 2
                if c % 2 == 0:
                    pend["t"] = knp.tile([P, 2, D], bf16, tag="knat",
                                         name=f"knat{c}")
                nc.scalar.activation(
                    out=pend["t"][:, c % 2], in_=py,
                    func=ACTF.Identity, bias=negmr, scale=rstd,
                )
                pend[pair] = pend["t"]

            def ktranspose(pair, pend):
                knat2 = pend.pop(pair)
                pos_sb = postp.tile([P, G6, 2 * P], bf16, tag="pos")
                nc.sync.dma_start(
                    out=pos_sb,
                    in_=posTb.rearrange("(s p) n -> p s n", p=P)[
                        :, :, pair * 2 * P:(pair + 1) * 2 * P],
                )
                pt = ps_t.tile([P, G6, 2 * P], bf16, tag="pt")
                for i in range(2):
                    for s in range(G6):
                        nc.tensor.transpose(
                            pt[:, s, i * P:(i + 1) * P],
                            knat2[:, i, s * P:(s + 1) * P], ident,
                        )
                for s in range(G6):
                    nc.gpsimd.scalar_tensor_tensor(
                        out=kT_sb[:, s, pair * 2 * P:(pair + 1) * 2 * P],
                        in0=pt[:, s], scalar=lnk_g_sb[:, s:s + 1],
                        in1=pos_sb[:, s], op0=ALU.mult, op1=ALU.add,
                    )

            pend = {}
            for c in range(MCH):
                kchunk(c, pend)
                if c >= 3 and c % 2 == 1:
                    ktranspose((c - 1) // 2 - 1, pend)  # one-pair lag
            ktranspose(MCH // 2 - 1, pend)

        # ---------------- attention: scores/exp (ACT-bound) + lagged av ----
        with (
            tc.tile_pool(name="atp", bufs=2) as atp,
            tc.tile_pool(name="rin", bufs=4) as rinp,
            tc.tile_pool(name="ps_s", bufs=2, space="PSUM") as ps_s,
            tc.tile_pool(name="ps_o", bufs=2, space="PSUM") as ps_o,
        ):
            def scores_group(at_h, h, mc0, w):
                p0 = DH * (h % 2)
                grp = h // 2
                ps = ps_s.tile([P, 3, S], f32, tag="ps3")
                for j in range(w):
                    mc = mc0 + j
                    nc.tensor.matmul(
                        ps[:, j],
                        kT_sb[p0:p0 + DH, grp, mc * P:(mc + 1) * P],
                        qT_sb[p0:p0 + DH, grp, :],
                        start=True, stop=True,
                    )
                nc.scalar.activation(
                    out=at_h[:, mc0:mc0 + w, :], in_=ps[:, :w],
                    func=ACTF.Exp, scale=SCALE,
                )

            def av_head(at_h, h):
                po = ps_o.tile([P, NCH, DH + 1], f32, tag="po")
                for qc in range(NCH):
                    for mc in range(MCH):
                        nc.tensor.matmul(
                            po[:, qc],
                            at_h[:, mc, qc * P:(qc + 1) * P],
                            vaug_sb[:, mc, h, :],
                            start=(mc == 0), stop=(mc == MCH - 1),
                        )
                    rinv = rinp.tile([P, 1], f32, tag="rin")
                    nc.vector.reciprocal(out=rinv, in_=po[:, qc, DH:DH + 1])
                    nc.gpsimd.tensor_scalar_mul(
                        out=onat_sb[:, qc, h], in0=po[:, qc, 0:DH],
                        scalar1=rinv,
                    )

            prev = None
            for h in range(H):
                at_h = atp.tile([P, MCH, S], bf16, tag="at", name=f"at{h}")
                for mc0, w in [(0, 3), (3, 3), (6, 3), (9, 3), (12, 3),
                               (15, 1)]:
                    scores_group(at_h, h, mc0, w)
                if prev is not None:
                    av_head(*prev)
                prev = (at_h, h)
            av_head(*prev)

        # transpose attention output for the output projection
        with tc.tile_pool(name="ps_t2", bufs=1, space="PSUM") as ps_t2:
            if True:
                for g in range(2):
                    pt = ps_t2.tile([P, G6, 2 * P], bf16, tag="pt2")
                    for i in range(2):
                        qc = g * 2 + i
                        src = onat_sb[:, qc].rearrange("p h d -> p (h d)")
                        for s in range(G6):
                            nc.tensor.transpose(
                                pt[:, s, i * P:(i + 1) * P],
                                src[:, s * P:(s + 1) * P], ident,
                            )
                    for s in range(G6):
                        nc.gpsimd.tensor_copy(
                            out=outT_sb[:, s, g * 2 * P:(g + 1) * 2 * P],
                            in_=pt[:, s],
                        )

        # ---------------- phase 3: out proj, gate, fuse, final LN ----------
        with (
            tc.tile_pool(name="ph3", bufs=1) as ph3,
            tc.tile_pool(name="fw", bufs=2) as fw,
            tc.tile_pool(name="st3", bufs=4) as st3,
            tc.tile_pool(name="ps_z", bufs=2, space="PSUM") as ps_z,
            tc.tile_pool(name="ps_t3", bufs=1, space="PSUM") as ps_t3,
        ):
            if has_gb:
                gb_sb = ph3.tile([1, D], bf16)
                nc.sync.dma_start(out=gb_sb, in_=gb.ap())
            if has_bo:
                bo_sb = ph3.tile([1, D], bf16)
                nc.sync.dma_start(out=bo_sb, in_=bo_a.ap())

            z_sb = ph3.tile([P, NCH, D], f32)
            zT_sb = ph3.tile([P, G6, S], bf16)
            gsig = ph3.tile([P, NCH, D], bf16)
            gbc = ph3.tile([P, D], f32)
            bbc = ph3.tile([P, D], f32)

            # broadcast final-LN gain/bias across partitions via K=1 matmul
            for dst, src_row in ((gbc, lnfw_sb), (bbc, lnfb_sb)):
                pb = ps_z.tile([P, D], f32, tag="pz")
                for o0, o1 in HALves:
                    nc.tensor.matmul(
                        pb[:, o0:o1], ones_f32, src_row[:, o0:o1],
                        start=True, stop=True,
                    )
                nc.vector.tensor_copy(out=dst, in_=pb)

            # z = attout @ woT' (+bo)
            for c in range(NCH):
                pz = ps_z.tile([P, D], f32, tag="pz")
                for o0, o1 in HALves:
                    for s in range(G6):
                        nc.tensor.matmul(
                            pz[:, o0:o1],
                            outT_sb[:, s, c * P:(c + 1) * P],
                            woT_sb[:, s, o0:o1],
                            start=(s == 0), stop=(not has_bo and s == G6 - 1),
                        )
                    if has_bo:
                        nc.tensor.matmul(
                            pz[:, o0:o1], ones_bf, bo_sb[:, o0:o1],
                            start=False, stop=True,
                        )
                nc.scalar.copy(out=z_sb[:, c], in_=pz)

            # zT for the gate matmul
            for g in range(2):
                pt = ps_t3.tile([P, G6, 2 * P], f32, tag="pt3")
                for i in range(2):
                    c = g * 2 + i
                    for s in range(G6):
                        nc.tensor.transpose(
                            pt[:, s, i * P:(i + 1) * P],
                            z_sb[:, c, s * P:(s + 1) * P], ident_f32,
                        )
                for s in range(G6):
                    nc.gpsimd.tensor_copy(
                        out=zT_sb[:, s, g * 2 * P:(g + 1) * 2 * P],
                        in_=pt[:, s],
                    )

            # gate = sigmoid([vis, z] @ gwT (+gb))
            for c in range(NCH):
                pg = ps_z.tile([P, D], f32, tag="pz")
                for o0, o1 in HALves:
                    for s in range(G6):
                        nc.tensor.matmul(
                            pg[:, o0:o1],
                            visT_sb[:, s, c * P:(c + 1) * P],
                            gwv_sb[:, s, o0:o1],
                            start=(s == 0), stop=False,
                        )
                    for s in range(G6):
                        nc.tensor.matmul(
                            pg[:, o0:o1],
                            zT_sb[:, s, c * P:(c + 1) * P],
                            gwz_sb[:, s, o0:o1],
                            start=False,
                            stop=(not has_gb and s == G6 - 1),
                        )
                    if has_gb:
                        nc.tensor.matmul(
                            pg[:, o0:o1], ones_bf, gb_sb[:, o0:o1],
                            start=False, stop=True,
                        )
                nc.scalar.activation(out=gsig[:, c], in_=pg, func=ACTF.Sigmoid)

            # fuse + final LN
            for c in range(NCH):
                dvz = fw.tile([P, D], f32, tag="dvz")
                nc.gpsimd.tensor_tensor(
                    out=dvz, in0=vis_sb[:, c], in1=z_sb[:, c], op=ALU.subtract,
                )
                fus = fw.tile([P, D], f32, tag="fus")
                nc.vector.tensor_tensor(out=fus, in0=gsig[:, c], in1=dvz,
                                        op=ALU.mult)
                nc.vector.tensor_tensor(out=fus, in0=fus, in1=z_sb[:, c],
                                        op=ALU.add)
                mv = stats(fus, st3)
                rstd = rstd_of(mv, st3, "fr")
                negmr = negmr_of(mv, rstd, st3, "fn")
                tnorm = fw.tile([P, D], f32, tag="tn")
                nc.scalar.activation(
                    out=tnorm, in_=fus, func=ACTF.Identity,
                    bias=negmr, scale=rstd,
                )
                nc.vector.tensor_tensor(out=tnorm, in0=tnorm, in1=gbc,
                                        op=ALU.mult)
                nc.gpsimd.tensor_tensor(out=tnorm, in0=tnorm, in1=bbc,
                                        op=ALU.add)
                nc.sync.dma_start(
                    out=out_rows.rearrange("(c p) o -> p c o", p=P)[:, c],
                    in_=tnorm,
                )

    nc.compile()
    return nc


def _prepare_in_maps(inputs):
    f32 = np.float32
    vis = np.asarray(inputs["visible_features"], f32)
    inf = np.asarray(inputs["infrared_features"], f32)
    wq = np.asarray(inputs["wq"], f32)
    bq = np.asarray(inputs["bq"], f32)
    lnq_w = np.asarray(inputs["lnq_w"], f32)
    lnq_b = np.asarray(inputs["lnq_b"], f32)
    wk = np.asarray(inputs["wk"], f32)
    bk = np.asarray(inputs["bk"], f32)
    lnk_w = np.asarray(inputs["lnk_w"], f32)
    lnk_b = np.asarray(inputs["lnk_b"], f32)
    wv = np.asarray(inputs["wv"], f32)
    bv = np.asarray(inputs["bv"], f32)
    lnv_w = np.asarray(inputs["lnv_w"], f32)
    lnv_b = np.asarray(inputs["lnv_b"], f32)
    pos = np.asarray(inputs["pos_emb"], f32)[:N]
    wo = np.asarray(inputs["wo"], f32)
    bo = np.asarray(inputs["bo"], f32)
    gw = np.asarray(inputs["gate_w"], f32)
    gb_ = np.asarray(inputs["gate_b"], f32)
    ln_w = np.asarray(inputs["ln_w"], f32)
    ln_b = np.asarray(inputs["ln_b"], f32)

    wqkvT = np.concatenate([wq.T, wk.T, wv.T], axis=1).astype(BF)
    bqkv = np.concatenate([bq, bk, bv])[None]
    woT = ((wo * lnv_w[None, :]).T).astype(BF)
    bo_a = (bo + wo @ lnv_b)[None]
    gwT = gw.T.astype(BF)
    lnq_g = np.ascontiguousarray(lnq_w.reshape(G6, P).T)
    lnq_b2 = np.ascontiguousarray(lnq_b.reshape(G6, P).T)
    lnk_g = np.ascontiguousarray(lnk_w.reshape(G6, P).T)
    lnf = np.stack([ln_w, ln_b])
    flags = (
        bool(np.any(bqkv)), bool(np.any(bo_a)), bool(np.any(gb_)),
    )

    in_maps = []
    for c in range(CORES):
        b, r0 = c // GROUP, (c % GROUP) * S
        m = {
            "xqT": np.ascontiguousarray(inf[b, r0:r0 + S].T).astype(BF),
            "xvTf": np.ascontiguousarray(vis[b].T).astype(BF),
            "visT_own": np.ascontiguousarray(vis[b, r0:r0 + S].T).astype(BF),
            "vis_nat": np.ascontiguousarray(vis[b, r0:r0 + S]),
            "posTb": np.ascontiguousarray(
                pos.T / SCALE + lnk_b[:, None]
            ).astype(BF),
            "wqkvT": np.ascontiguousarray(wqkvT),
            "woT": np.ascontiguousarray(woT),
            "gwT": np.ascontiguousarray(gwT),
            "lnq_g": lnq_g,
            "lnq_b": lnq_b2,
            "lnk_g": lnk_g,
            "lnf": lnf,
        }
        if flags[0]:
            m["bqkv"] = np.ascontiguousarray(bqkv).astype(BF)
        if flags[1]:
            m["bo_a"] = np.ascontiguousarray(bo_a).astype(BF)
        if flags[2]:
            m["gb"] = np.ascontiguousarray(gb_[None]).astype(BF)
        in_maps.append(m)
    return in_maps, flags


def kernel(trace=False, **inputs):
    from concourse.bass_utils import run_bass_kernel_spmd

    in_maps, flags = _prepare_in_maps(inputs)
    key = ("nc",) + flags
    if key not in _CACHE:
        _CACHE[key] = _build(*flags)
    nc = _CACHE[key]
    _CACHE["nc"] = nc
    res = run_bass_kernel_spmd(
        nc, in_maps, core_ids=list(range(CORES)), trace=trace
    )
    out = np.empty((B, N, D), np.float32)
    for c in range(CORES):
        b, r0 = c // GROUP, (c % GROUP) * S
        out[b, r0:r0 + S] = res.results[c]["out_rows"]
    _CACHE["last_result"] = res
    return out
